# revision 1
# baseline (speedup 1.0000x reference)
"""Trainium2 Bass kernel for a dense transformer block (B=4, T=2048, D=1024, H=16).

Sharding: 8 cores = 4 batches x 2 head-halves.  Each core computes LN1
(mean fold in weights, rstd pre-scaled into the bf16 matmul operand),
Q/K/V for its 8 heads over all 2048 tokens, causal attention with
hb-paired [128,1024] exp groups (denominator via a ones-column in V),
then a row-parallel Wo partial that is pairwise ReduceScattered so each
core ends up with 1024 tokens for LN2 + MLP.  All matmul operands are
bf16 (fp32 PSUM accumulate); weights are SBUF-resident or streamed
exactly once per use.  MLP for the first 512-token chunk is interleaved
into the last attention tile; gelu uses the tanh approximation (stays
near the exp activation table).

Host-side prep (numpy only): fold LayerNorm gains/means into the
projection weights, cast to bf16, lay out every tensor partition-major.
"""

import os
import sys

for _p in ("/opt/trn_rl_repo", "/root/.axon_site/_ro/trn_rl_repo"):
    if os.path.isdir(_p) and _p not in sys.path:
        sys.path.append(_p)

import numpy as np

import concourse.bass as bass
import concourse.tile as tile
from concourse import bacc, mybir
from concourse.bass_utils import run_bass_kernel_spmd

AF = mybir.ActivationFunctionType
ALU = mybir.AluOpType
FP32 = mybir.dt.float32
FP32R = mybir.dt.float32r
BF16 = mybir.dt.bfloat16

B, T, D, H = 4, 2048, 1024, 16
HD = D // H          # 64
DFF = 4 * D          # 4096
P = 128
DK = D // P          # 8   D k-tiles
NT = T // 512        # 4   512-token tiles
HC = H // 2          # 8   local heads per core
DQ = HC * HD         # 512 local qkv width
NOT = DQ // P        # 4   local qkv feature tiles (head pairs)
FFT = DFF // P       # 32  DFF tiles
FH = FFT // 2        # 16  half-FFT group
TOWN = T // 2        # 1024 own tokens after ReduceScatter
EPS = 1e-5
SCALE = 1.0 / 8.0    # 1/sqrt(HD)


def build_program():
    nc = bacc.Bacc("TRN2", target_bir_lowering=False, debug=False)

    # ---- DRAM I/O (all partition-major contiguous) ----
    xT = nc.dram_tensor("xT", [P, DK, T], BF16, kind="ExternalInput")
    xTo = nc.dram_tensor("xTo", [P, DK, TOWN], BF16, kind="ExternalInput")
    wqk = nc.dram_tensor("wqk", [P, 2, NOT, DK, P], BF16, kind="ExternalInput")
    wv = nc.dram_tensor("wv", [P, DK, DQ], BF16, kind="ExternalInput")
    wo = nc.dram_tensor("wo", [P, NOT, D], BF16, kind="ExternalInput")
    w1 = nc.dram_tensor("w1", [FFT, P, DK, P], BF16, kind="ExternalInput")
    w2 = nc.dram_tensor("w2", [DK, P, FFT, P], BF16, kind="ExternalInput")
    cqk = nc.dram_tensor("cqk", [P, 2 * NOT], FP32, kind="ExternalInput")
    cvb = nc.dram_tensor("cvb", [P, DQ], BF16, kind="ExternalInput")
    bo = nc.dram_tensor("bo", [P, DK], FP32, kind="ExternalInput")
    c1 = nc.dram_tensor("c1", [P, FFT], FP32, kind="ExternalInput")
    b2 = nc.dram_tensor("b2", [P, DK], FP32, kind="ExternalInput")
    masks = nc.dram_tensor("masks", [P, 896], BF16, kind="ExternalInput")
    out = nc.dram_tensor("out", [P, DK, TOWN], FP32, kind="ExternalOutput")

    with tile.TileContext(nc) as tc:
        with (
            tc.tile_pool(name="mp", bufs=1) as mp,
            tc.tile_pool(name="psum", bufs=1, space="PSUM") as psum,
            tc.tile_pool(name="dram", bufs=1, space="DRAM") as dram,
        ):
            # ---- persistent small tiles ----
            ones_b = mp.tile([P, 1], BF16)
            nc.vector.memset(ones_b, 1.0)
            eps_sb = mp.tile([1, 1], FP32)
            nc.vector.memset(eps_sb, EPS)
            # prefetch first x tile ahead of the big weight loads
            xt0 = mp.tile([P, DK, 512], BF16, tag="xtrot", bufs=2, name="xt0")
            nc.sync.dma_start(xt0, xT[:, :, 0:512])
            # ---- resident weights (wqk first: needed ~10us in) ----
            wqk_sb = mp.tile([P, 2, NOT, DK, P], BF16)
            nc.sync.dma_start(wqk_sb, wqk[:, :, :, :, :])
            wv_sb = mp.tile([P, DK, DQ], BF16)
            nc.sync.dma_start(wv_sb, wv[:, :, :])
            cqk_sb = mp.tile([P, 2 * NOT], FP32)
            nc.sync.dma_start(cqk_sb, cqk[:, :])
            cvb_sb = mp.tile([P, DQ], BF16)
            nc.sync.dma_start(cvb_sb, cvb[:, :])
            wo_sb = mp.tile([P, NOT, D], BF16)
            nc.sync.dma_start(wo_sb, wo[:, :, :])
            bo_sb = mp.tile([P, DK], FP32)
            nc.sync.dma_start(bo_sb, bo[:, :])
            c1_sb = mp.tile([P, FFT], FP32)
            nc.sync.dma_start(c1_sb, c1[:, :])
            b2_sb = mp.tile([P, DK], FP32)
            nc.sync.dma_start(b2_sb, b2[:, :])
            masks_sb = mp.tile([P, 896], BF16)
            nc.sync.dma_start(masks_sb, masks[:, :])

            # ---- resident activations ----
            kT_sb = mp.tile([P, NOT, T], BF16)
            v_sb = mp.tile([P, HC, T // P, HD + 1], BF16)
            for h in range(HC):
                nc.vector.memset(v_sb[:, h, :, HD:HD + 1], 1.0)
            x2_sb = mp.tile([P, 2, DK, 512], BF16)

            # DRAM scratch for the pairwise ReduceScatter
            rs_in = [dram.tile([2, DK, P, 512], BF16, name=f"rsin{i}",
                               tag=f"rsin{i}") for i in range(2)]
            rs_out = [dram.tile([DK, P, 512], BF16, name=f"rsout{i}",
                                tag=f"rsout{i}") for i in range(2)]

            qcur_t = [None] * NT
            x2s_t = [None] * 2
            m_t = [None] * 2

            def ln_stats(src3, nblk):
                """rstd of columns of src3 ([P, nblk, 512] bf16) -> psum
                [P,512] fp32 broadcast (acc-tag tile; released after reads)."""
                s_ps = psum.tile([1, 512], FP32, tag="acc", bufs=2)
                for kt in range(nblk):
                    nc.tensor.matmul(s_ps, ones_b, src3[:, kt, :],
                                     start=(kt == 0), stop=(kt == nblk - 1))
                q_ps = psum.tile([1, 512], FP32, tag="acc", bufs=2)
                for kt in range(nblk):
                    xsq = mp.tile([P, 512], BF16, tag="xsq", bufs=1)
                    eng = nc.vector if kt % 2 == 0 else nc.gpsimd
                    eng.tensor_mul(xsq, src3[:, kt, :], src3[:, kt, :])
                    nc.tensor.matmul(q_ps, ones_b, xsq,
                                     start=(kt == 0), stop=(kt == nblk - 1))
                trow = mp.tile([1, 512], FP32, tag="trow", bufs=1)
                vrow = mp.tile([1, 512], FP32, tag="vrow", bufs=1)
                rs_row = mp.tile([1, 512], FP32, tag="rsrow", bufs=1)
                nc.vector.tensor_scalar(trow, s_ps, 1.0 / D, None, ALU.mult)
                nc.vector.tensor_mul(trow, trow, trow)
                nc.vector.scalar_tensor_tensor(vrow, q_ps, 1.0 / D, trow,
                                               ALU.mult, ALU.subtract)
                nc.scalar.activation(vrow, vrow, AF.Sqrt, bias=eps_sb)
                nc.vector.reciprocal(rs_row, vrow)
                rsb = mp.tile([P, 512], FP32, tag="rsb", bufs=2)
                nc.gpsimd.partition_broadcast(rsb, rs_row)
                return rsb

            def phase_ab(tt):
                ts5 = slice(tt * 512, (tt + 1) * 512)
                if tt == 0:
                    xt = xt0
                else:
                    xt = mp.tile([P, DK, 512], BF16, tag="xtrot", bufs=2)
                    nc.sync.dma_start(xt, xT[:, :, ts5])
                rsb = ln_stats(xt, DK)
                # pre-scaled LN1 output (bf16 matmul operand)
                h_sb = mp.tile([P, DK, 512], BF16, tag="h", bufs=1)
                for kt in range(DK):
                    eng = nc.vector if kt % 2 == 0 else nc.gpsimd
                    eng.tensor_mul(h_sb[:, kt, :], xt[:, kt, :], rsb)
                # q/k projections
                qcur = mp.tile([P, NOT, 512], BF16, tag="qcur", bufs=2)
                qcur_t[tt] = qcur
                for proj in range(2):
                    for ot in range(NOT):
                        pp = psum.tile([P, 512], FP32, tag="acc", bufs=2)
                        for kt in range(DK):
                            nc.tensor.matmul(pp, wqk_sb[:, proj, ot, kt, :],
                                             h_sb[:, kt, :],
                                             start=(kt == 0),
                                             stop=(kt == DK - 1))
                        dest = (qcur[:, ot, :] if proj == 0
                                else kT_sb[:, ot, ts5])
                        nc.vector.tensor_scalar(
                            dest, pp,
                            cqk_sb[:, proj * NOT + ot:proj * NOT + ot + 1],
                            None, ALU.add)
                # v projection (token-major)
                for st in range(4):
                    pp = psum.tile([P, 512], FP32, tag="acc", bufs=2)
                    for kt in range(DK):
                        nc.tensor.matmul(
                            pp, h_sb[:, kt, st * P:(st + 1) * P],
                            wv_sb[:, kt, :],
                            start=(kt == 0), stop=(kt == DK - 1))
                    nc.vector.tensor_tensor(
                        v_sb[:, :, tt * 4 + st, 0:HD],
                        pp.rearrange("p (h e) -> p h e", h=HC),
                        cvb_sb.rearrange("p (h e) -> p h e", h=HC), ALU.add)

            def phase_e(i):
                """x2 = x + Wo-out + bo for 512-token chunk i, then LN2."""
                x2 = x2_sb[:, i]
                nc.gpsimd.dma_start(x2, rs_out[i].rearrange("k p t -> p k t"))
                xo = mp.tile([P, DK, 512], BF16, tag="xtrot", bufs=2)
                nc.sync.dma_start(xo, xTo[:, :, i * 512:(i + 1) * 512])
                for kt in range(DK):
                    nc.vector.scalar_tensor_tensor(
                        x2[:, kt, :], x2[:, kt, :], bo_sb[:, kt:kt + 1],
                        xo[:, kt, :], ALU.add, ALU.add)
                rsb = ln_stats(x2, DK)
                x2s = mp.tile([P, DK, 512], BF16, tag="x2s", bufs=1)
                x2s_t[i] = x2s
                for kt in range(DK):
                    eng = nc.vector if kt % 2 == 0 else nc.gpsimd
                    eng.tensor_mul(x2s[:, kt, :], x2[:, kt, :], rsb)

            def fc1_piece(i, fs, fe):
                """fc1+gelu for global ffts [fs, fe) of chunk i."""
                x2s = x2s_t[i]
                for fft in range(fs, fe):
                    w1t = mp.tile([P, DK, P], BF16, tag="w1t", bufs=3)
                    nc.sync.dma_start(w1t, w1[fft])
                    pp = psum.tile([P, 512], FP32, tag="acc", bufs=2)
                    for kt in range(DK):
                        nc.tensor.matmul(pp, w1t[:, kt, :], x2s[:, kt, :],
                                         start=(kt == 0), stop=(kt == DK - 1))
                    nc.scalar.activation(m_t[i][:, fft % FH, :], pp,
                                         AF.Gelu_apprx_tanh,
                                         bias=c1_sb[:, fft:fft + 1])

            def fc2_half(i, half, acc2):
                """fc2 partial over ffts [half*FH, (half+1)*FH)."""
                x2 = x2_sb[:, i]
                for ot in range(DK):
                    w2t = mp.tile([P, FH, P], BF16, tag="w2t", bufs=2)
                    nc.sync.dma_start(w2t, w2[ot, :, half * FH:(half + 1) * FH])
                    pp = psum.tile([P, 512], FP32, tag="acc", bufs=2)
                    for ff in range(FH):
                        nc.tensor.matmul(pp, w2t[:, ff, :],
                                         m_t[i][:, ff, :],
                                         start=(ff == 0), stop=(ff == FH - 1))
                    if half == 0:
                        # stash (fc2A + b2 + x2) in bf16
                        nc.vector.scalar_tensor_tensor(
                            acc2[:, ot, :], pp, b2_sb[:, ot:ot + 1],
                            x2[:, ot, :], ALU.add, ALU.add)
                    else:
                        ot_t = mp.tile([P, 512], FP32, tag="outt", bufs=1)
                        nc.vector.tensor_tensor(ot_t, pp, acc2[:, ot, :],
                                                ALU.add)
                        nc.sync.dma_start(
                            out[:, ot, i * 512:(i + 1) * 512], ot_t)

            def attn(qt, mlp_cb=None):
                """attention + Wo partial for q-tile qt.  mlp_cb(pt) lets the
                caller interleave MLP pieces between pt bands."""
                nkt = 4 * qt + 4
                qv = qcur_t[qt]
                ysb = mp.tile([P, NOT, 512], BF16, tag="ysb", bufs=2)
                for pt in range(NOT):
                    y_ps = [psum.tile([HD + 1, 512], FP32, name=f"yps{hb}",
                                      tag=f"y{hb}", bufs=1) for hb in range(2)]
                    for kt in range(nkt):
                        ksl = slice(kt * P, (kt + 1) * P)
                        sg = psum.tile([P, 1024], FP32, tag="sg", bufs=2)
                        nc.tensor.matmul(sg[:, 0:512],
                                         kT_sb[0:HD, pt, ksl],
                                         qv[0:HD, pt, :],
                                         start=True, stop=True)
                        nc.tensor.matmul(sg[:, 512:1024],
                                         kT_sb[HD:P, pt, ksl],
                                         qv[HD:P, pt, :],
                                         start=True, stop=True)
                        pe = mp.tile([P, 1024], BF16, tag="pexp", bufs=3)
                        nc.scalar.activation(pe, sg, AF.Exp, scale=SCALE)
                        j = kt - 4 * qt
                        if j >= 0:
                            moff = 384 - P * j
                            nc.vector.tensor_mul(
                                pe[:, 0:512], pe[:, 0:512],
                                masks_sb[:, moff:moff + 512])
                            nc.vector.tensor_mul(
                                pe[:, 512:1024], pe[:, 512:1024],
                                masks_sb[:, moff:moff + 512])
                        nc.tensor.matmul(y_ps[0], v_sb[:, 2 * pt, kt, :],
                                         pe[:, 0:512],
                                         start=(kt == 0), stop=(kt == nkt - 1))
                        nc.tensor.matmul(y_ps[1], v_sb[:, 2 * pt + 1, kt, :],
                                         pe[:, 512:1024],
                                         start=(kt == 0), stop=(kt == nkt - 1))
                    for hb in range(2):
                        den = mp.tile([HD + 1, 512], FP32, tag="den", bufs=1)
                        nc.vector.reciprocal(den[HD:HD + 1, :],
                                             y_ps[hb][HD:HD + 1, :])
                        rec = mp.tile([1, 512], FP32, tag="rec", bufs=1)
                        nc.sync.dma_start(rec, den[HD:HD + 1, :])
                        rb = mp.tile([HD, 512], FP32, tag="rb", bufs=2)
                        nc.gpsimd.partition_broadcast(rb, rec)
                        if hb == 0:
                            nc.vector.tensor_mul(ysb[0:HD, pt, :],
                                                 y_ps[hb][0:HD, :], rb)
                        else:
                            yst = mp.tile([HD, 512], BF16, tag="yst", bufs=1)
                            nc.vector.tensor_mul(yst, y_ps[hb][0:HD, :], rb)
                            nc.sync.dma_start(ysb[HD:P, pt, :], yst)
                    if mlp_cb is not None:
                        mlp_cb(pt)
                # Wo partials
                ast = mp.tile([P, DK, 512], BF16, tag="ast", bufs=1)
                for ot in range(DK):
                    pp = psum.tile([P, 512], FP32, tag="acc", bufs=2)
                    for pt in range(NOT):
                        nc.tensor.matmul(pp, wo_sb[:, pt, ot * P:(ot + 1) * P],
                                         ysb[:, pt, :],
                                         start=(pt == 0), stop=(pt == NOT - 1))
                    nc.vector.tensor_copy(ast[:, ot, :], pp)
                nc.gpsimd.dma_start(
                    rs_in[qt % 2].rearrange("s k p t -> s p k t")[qt // 2],
                    ast)
                if qt >= 2:
                    i = qt - 2
                    nc.gpsimd.collective_compute(
                        "ReduceScatter", ALU.add,
                        replica_groups=[[0, 1], [2, 3], [4, 5], [6, 7]],
                        ins=[rs_in[i].opt()], outs=[rs_out[i].opt()])

            # ==================== program ====================
            for tt in range(NT):
                phase_ab(tt)
                if 1 <= tt <= 2:
                    attn(tt - 1)
            attn(2)          # fires ReduceScatter #0
            m_t[0] = mp.tile([P, FH, 512], BF16, tag="m", bufs=1, name="m0")
            acc2_0 = mp.tile([P, DK, 512], BF16, tag="acc2", bufs=1,
                             name="acc2_0")

            def mlp0_cb(pt):
                if pt == 0:
                    phase_e(0)
                elif pt == 1:
                    fc1_piece(0, 0, FH)
                elif pt == 2:
                    fc2_half(0, 0, acc2_0)
                else:
                    fc1_piece(0, FH, FFT)

            attn(3, mlp_cb=mlp0_cb)   # fires ReduceScatter #1
            fc2_half(0, 1, acc2_0)
            phase_e(1)
            m_t[1] = mp.tile([P, FH, 512], BF16, tag="m", bufs=1, name="m1")
            acc2_1 = mp.tile([P, DK, 512], BF16, tag="acc2", bufs=1,
                             name="acc2_1")
            fc1_piece(1, 0, FH)
            fc2_half(1, 0, acc2_1)
            fc1_piece(1, FH, FFT)
            fc2_half(1, 1, acc2_1)

    nc.compile()
    return nc


_NC_CACHE = None


def _get_nc():
    global _NC_CACHE
    if _NC_CACHE is None:
        _NC_CACHE = build_program()
    return _NC_CACHE


def prep_in_maps(x, ln1_g, ln1_b, ln2_g, ln2_b, Wq, bq, Wk, bk, Wv, bv,
                 Wo, bo, W1, b1, W2, b2):
    import ml_dtypes
    f32 = np.float32
    bf16 = ml_dtypes.bfloat16
    x = np.asarray(x, f32)
    ln1_g, ln1_b = np.asarray(ln1_g, f32), np.asarray(ln1_b, f32)
    ln2_g, ln2_b = np.asarray(ln2_g, f32), np.asarray(ln2_b, f32)
    Wq, Wk, Wv, Wo = (np.asarray(a, f32) for a in (Wq, Wk, Wv, Wo))
    W1, W2 = np.asarray(W1, f32), np.asarray(W2, f32)
    bq, bk, bv, bo_, b1, b2_ = (np.asarray(a, f32)
                                for a in (bq, bk, bv, bo, b1, b2))

    # fold LN gain AND the mean subtraction (a rank-1 correction) into W:
    # (x - mu) * g @ W  =  x @ (g*W - colsum(g*W)/D)
    Wqg = ln1_g[:, None] * Wq
    Wkg = ln1_g[:, None] * Wk
    Wvg = ln1_g[:, None] * Wv
    Wqg = Wqg - Wqg.sum(0, keepdims=True) / D
    Wkg = Wkg - Wkg.sum(0, keepdims=True) / D
    Wvg = Wvg - Wvg.sum(0, keepdims=True) / D
    cq_full = ln1_b @ Wq + bq
    ck_full = ln1_b @ Wk + bk
    cv_full = ln1_b @ Wv + bv
    W1g = ln2_g[:, None] * W1
    W1g = W1g - W1g.sum(0, keepdims=True) / D
    c1_full = ln2_b @ W1 + b1

    w1_t = np.ascontiguousarray(
        W1g.reshape(DK, P, FFT, P).transpose(2, 1, 0, 3)).astype(bf16)
    w2_t = np.ascontiguousarray(
        W2.reshape(FFT, P, DK, P).transpose(2, 1, 0, 3)).astype(bf16)
    c1_t = np.ascontiguousarray(c1_full.reshape(FFT, P).T)      # [P,FFT]
    b2_t = np.ascontiguousarray(b2_.reshape(DK, P).T)           # [P,DK]
    bo_t = np.ascontiguousarray(bo_.reshape(DK, P).T)           # [P,DK]

    kk = np.arange(P)[:, None]
    cc = np.arange(896)[None, :]
    mk = (kk + 384 <= cc).astype(bf16)                          # [P,896]

    in_maps = []
    for c in range(8):
        b_idx, hh = c // 2, c % 2
        sl = slice(DQ * hh, DQ * hh + DQ)
        xT_c = np.ascontiguousarray(
            x[b_idx].T.reshape(DK, P, T).transpose(1, 0, 2)).astype(bf16)
        wq_c, wk_c = Wqg[:, sl], Wkg[:, sl]
        wqk_t = np.ascontiguousarray(
            np.stack([wq_c, wk_c]).reshape(2, DK, P, NOT, P)
            .transpose(2, 0, 3, 1, 4)).astype(bf16)             # [P,2,NOT,DK,P]
        wv_t = np.ascontiguousarray(
            Wvg[:, sl].reshape(DK, P, DQ).transpose(1, 0, 2)).astype(bf16)
        wo_t = np.ascontiguousarray(
            Wo[sl, :].reshape(NOT, P, D).transpose(1, 0, 2)).astype(bf16)
        cq_t = cq_full[sl].reshape(NOT, P).T                    # [P,NOT]
        ck_t = ck_full[sl].reshape(NOT, P).T
        in_maps.append({
            "xT": xT_c,
            "xTo": np.ascontiguousarray(
                xT_c[:, :, hh * TOWN:(hh + 1) * TOWN]),
            "wqk": wqk_t,
            "wv": wv_t,
            "wo": wo_t,
            "w1": w1_t,
            "w2": w2_t,
            "cqk": np.ascontiguousarray(np.concatenate([cq_t, ck_t], axis=1)),
            "cvb": np.broadcast_to(
                cv_full[sl][None, :], (P, DQ)).astype(bf16),
            "bo": bo_t,
            "c1": c1_t,
            "b2": b2_t,
            "masks": mk,
        })
    return in_maps


def assemble_output(results):
    out = np.empty((B, T, D), np.float32)
    for c in range(8):
        b_idx, hh = c // 2, c % 2
        o = results[c]["out"]                                   # [P,DK,TOWN]
        out[b_idx, hh * TOWN:(hh + 1) * TOWN, :] = (
            o.transpose(2, 1, 0).reshape(TOWN, D))
    return out


def kernel(**inputs):
    nc = _get_nc()
    in_maps = prep_in_maps(**inputs)
    res = run_bass_kernel_spmd(nc, in_maps, list(range(8)))
    return assemble_output(res.results)



# revision 18
# speedup vs baseline: 1.0791x; 1.0791x over previous
"""Trainium2 Bass kernel for a dense transformer block (B=4, T=2048, D=1024, H=16).

Sharding: 8 cores = 4 batches x 2 head-halves.  Each core computes LN1
(mean fold in weights, rstd pre-scaled into the bf16 matmul operand),
Q/K/V for its 8 heads over all 2048 tokens, causal attention with
hb-paired [128,1024] exp groups (denominator via a ones-column in V),
then a row-parallel Wo partial that is pairwise ReduceScattered so each
core ends up with 1024 tokens for LN2 + MLP.  All matmul operands are
bf16 (fp32 PSUM accumulate); weights are SBUF-resident or streamed
exactly once per use.  MLP for the first 512-token chunk is interleaved
into the last attention tile; gelu uses the tanh approximation (stays
near the exp activation table).

Host-side prep (numpy only): fold LayerNorm gains/means into the
projection weights, cast to bf16, lay out every tensor partition-major.
"""

import os
import sys

for _p in ("/opt/trn_rl_repo", "/root/.axon_site/_ro/trn_rl_repo"):
    if os.path.isdir(_p) and _p not in sys.path:
        sys.path.append(_p)

import numpy as np

import concourse.bass as bass
import concourse.tile as tile
from concourse import bacc, mybir
from concourse.bass_utils import run_bass_kernel_spmd

AF = mybir.ActivationFunctionType
ALU = mybir.AluOpType
FP32 = mybir.dt.float32
FP32R = mybir.dt.float32r
BF16 = mybir.dt.bfloat16

B, T, D, H = 4, 2048, 1024, 16
HD = D // H          # 64
DFF = 4 * D          # 4096
P = 128
DK = D // P          # 8   D k-tiles
NT = T // 512        # 4   512-token tiles
HC = H // 2          # 8   local heads per core
DQ = HC * HD         # 512 local qkv width
NOT = DQ // P        # 4   local qkv feature tiles (head pairs)
FFT = DFF // P       # 32  DFF tiles
FH = FFT // 2        # 16  half-FFT group
TOWN = T // 2        # 1024 own tokens after ReduceScatter
EPS = 1e-5
SCALE = 1.0 / 8.0    # 1/sqrt(HD)


def build_program():
    nc = bacc.Bacc("TRN2", target_bir_lowering=False, debug=False)

    # ---- DRAM I/O (all partition-major contiguous) ----
    xT = nc.dram_tensor("xT", [P, DK, T], BF16, kind="ExternalInput")
    xTo = nc.dram_tensor("xTo", [P, DK, TOWN], BF16, kind="ExternalInput")
    wqk = nc.dram_tensor("wqk", [P, 2, NOT, DK, P], BF16, kind="ExternalInput")
    wv = nc.dram_tensor("wv", [P, DK, DQ], BF16, kind="ExternalInput")
    wo = nc.dram_tensor("wo", [P, NOT, D], BF16, kind="ExternalInput")
    w1 = nc.dram_tensor("w1", [FFT, P, DK, P], BF16, kind="ExternalInput")
    w2 = nc.dram_tensor("w2", [DK, P, FFT, P], BF16, kind="ExternalInput")
    cqk = nc.dram_tensor("cqk", [P, 2 * NOT], FP32, kind="ExternalInput")
    cvb = nc.dram_tensor("cvb", [P, DQ], BF16, kind="ExternalInput")
    bo = nc.dram_tensor("bo", [P, DK], FP32, kind="ExternalInput")
    c1 = nc.dram_tensor("c1", [P, FFT], FP32, kind="ExternalInput")
    b2 = nc.dram_tensor("b2", [P, DK], FP32, kind="ExternalInput")
    masks = nc.dram_tensor("masks", [P, 896], BF16, kind="ExternalInput")
    out = nc.dram_tensor("out", [P, DK, TOWN], FP32, kind="ExternalOutput")

    with tile.TileContext(nc) as tc:
        with (
            tc.tile_pool(name="mp", bufs=1) as mp,
            tc.tile_pool(name="psum", bufs=1, space="PSUM") as psum,
            tc.tile_pool(name="dram", bufs=1, space="DRAM") as dram,
        ):
            # ---- persistent small tiles ----
            ones_b = mp.tile([P, 1], BF16)
            nc.vector.memset(ones_b, 1.0)
            ones128 = mp.tile([P, P], BF16)
            nc.vector.memset(ones128, 1.0)
            eps_sb = mp.tile([P, 1], FP32)
            nc.vector.memset(eps_sb, EPS)
            # prefetch first x tile ahead of the big weight loads
            xt0 = mp.tile([P, DK, 512], BF16, tag="xtrot", bufs=2, name="xt0")
            nc.sync.dma_start(xt0, xT[:, :, 0:512])
            # ---- resident weights (wqk first: needed ~10us in) ----
            wqk_sb = mp.tile([P, 2, NOT, DK, P], BF16)
            nc.sync.dma_start(wqk_sb, wqk[:, :, :, :, :])
            wv_sb = mp.tile([P, DK, DQ], BF16)
            nc.sync.dma_start(wv_sb, wv[:, :, :])
            cqk_sb = mp.tile([P, 2 * NOT], FP32)
            nc.sync.dma_start(cqk_sb, cqk[:, :])
            cvb_sb = mp.tile([P, DQ], BF16)
            nc.sync.dma_start(cvb_sb, cvb[:, :])
            wo_sb = mp.tile([P, NOT, D], BF16)
            nc.sync.dma_start(wo_sb, wo[:, :, :])
            bo_sb = mp.tile([P, DK], FP32)
            nc.sync.dma_start(bo_sb, bo[:, :])
            c1_sb = mp.tile([P, FFT], FP32)
            nc.sync.dma_start(c1_sb, c1[:, :])
            b2_sb = mp.tile([P, DK], FP32)
            nc.sync.dma_start(b2_sb, b2[:, :])
            masks_sb = mp.tile([P, 896], BF16)
            nc.sync.dma_start(masks_sb, masks[:, :])

            # ---- resident activations ----
            kT_sb = mp.tile([P, NOT, T], BF16)
            v_sb = mp.tile([P, HC, T // P, HD + 1], BF16)
            for h in range(HC):
                nc.vector.memset(v_sb[:, h, :, HD:HD + 1], 1.0)
            x2_sb = mp.tile([P, 2, DK, 512], BF16)

            # DRAM scratch for the pairwise ReduceScatter
            rs_in = [dram.tile([2, DK, P, 512], BF16, name=f"rsin{i}",
                               tag=f"rsin{i}") for i in range(2)]
            rs_out = [dram.tile([DK, P, 512], BF16, name=f"rsout{i}",
                                tag=f"rsout{i}") for i in range(2)]

            qcur_t = [None] * NT
            x2s_t = [None] * 2

            def ln_stats(src3, nblk):
                """rstd of columns of src3 ([P, nblk, 512] bf16) -> sbuf
                [P,512] fp32, already broadcast across partitions (the sum
                matmuls use a ones [P,P] stationary so every partition of the
                PSUM result holds the column sum)."""
                s_ps = psum.tile([P, 512], FP32, tag="acc", bufs=2)
                for kt in range(nblk):
                    nc.tensor.matmul(s_ps, ones128, src3[:, kt, :],
                                     start=(kt == 0), stop=(kt == nblk - 1))
                q_ps = psum.tile([P, 512], FP32, tag="acc", bufs=2)
                for kt in range(nblk):
                    xsq = mp.tile([P, 512], BF16, tag="xsq", bufs=1)
                    eng = nc.vector if kt % 2 == 0 else nc.gpsimd
                    eng.tensor_mul(xsq, src3[:, kt, :], src3[:, kt, :])
                    nc.tensor.matmul(q_ps, ones128, xsq,
                                     start=(kt == 0), stop=(kt == nblk - 1))
                trow = mp.tile([P, 512], FP32, tag="trow", bufs=1)
                vrow = mp.tile([P, 512], FP32, tag="vrow", bufs=1)
                nc.vector.tensor_scalar(trow, s_ps, 1.0 / D, None, ALU.mult)
                nc.vector.tensor_mul(trow, trow, trow)
                nc.vector.scalar_tensor_tensor(vrow, q_ps, 1.0 / D, trow,
                                               ALU.mult, ALU.subtract)
                nc.scalar.activation(vrow, vrow, AF.Sqrt, bias=eps_sb)
                rsb = mp.tile([P, 512], FP32, tag="rsb", bufs=2)
                nc.vector.reciprocal(rsb, vrow)
                return rsb

            def phase_ab(tt):
                ts5 = slice(tt * 512, (tt + 1) * 512)
                if tt == 0:
                    xt = xt0
                else:
                    xt = mp.tile([P, DK, 512], BF16, tag="xtrot", bufs=2)
                    nc.sync.dma_start(xt, xT[:, :, ts5])
                rsb = ln_stats(xt, DK)
                # pre-scaled LN1 output (bf16 matmul operand)
                h_sb = mp.tile([P, DK, 512], BF16, tag="h", bufs=1)
                for kt in range(DK):
                    eng = nc.vector if kt % 2 == 0 else nc.gpsimd
                    eng.tensor_mul(h_sb[:, kt, :], xt[:, kt, :], rsb)
                # q/k projections
                qcur = mp.tile([P, NOT, 512], BF16, tag="qcur", bufs=2)
                qcur_t[tt] = qcur
                for proj in range(2):
                    for ot in range(NOT):
                        pp = psum.tile([P, 512], FP32, tag="acc", bufs=2)
                        for kt in range(DK):
                            nc.tensor.matmul(pp, wqk_sb[:, proj, ot, kt, :],
                                             h_sb[:, kt, :],
                                             start=(kt == 0),
                                             stop=(kt == DK - 1))
                        dest = (qcur[:, ot, :] if proj == 0
                                else kT_sb[:, ot, ts5])
                        nc.vector.tensor_scalar(
                            dest, pp,
                            cqk_sb[:, proj * NOT + ot:proj * NOT + ot + 1],
                            None, ALU.add)
                # v projection (token-major)
                for st in range(4):
                    pp = psum.tile([P, 512], FP32, tag="acc", bufs=2)
                    for kt in range(DK):
                        nc.tensor.matmul(
                            pp, h_sb[:, kt, st * P:(st + 1) * P],
                            wv_sb[:, kt, :],
                            start=(kt == 0), stop=(kt == DK - 1))
                    nc.vector.tensor_tensor(
                        v_sb[:, :, tt * 4 + st, 0:HD],
                        pp.rearrange("p (h e) -> p h e", h=HC),
                        cvb_sb.rearrange("p (h e) -> p h e", h=HC), ALU.add)

            def phase_e(i):
                """x2 = x + Wo-out + bo for 512-token chunk i, then LN2."""
                x2 = x2_sb[:, i]
                nc.gpsimd.dma_start(x2, rs_out[i].rearrange("k p t -> p k t"))
                xo = mp.tile([P, DK, 512], BF16, tag="xtrot", bufs=2)
                nc.sync.dma_start(xo, xTo[:, :, i * 512:(i + 1) * 512])
                for kt in range(DK):
                    nc.vector.scalar_tensor_tensor(
                        x2[:, kt, :], x2[:, kt, :], bo_sb[:, kt:kt + 1],
                        xo[:, kt, :], ALU.add, ALU.add)
                rsb = ln_stats(x2, DK)
                x2s = mp.tile([P, DK, 512], BF16, tag="x2s", bufs=1)
                x2s_t[i] = x2s
                for kt in range(DK):
                    eng = nc.vector if kt % 2 == 0 else nc.gpsimd
                    eng.tensor_mul(x2s[:, kt, :], x2[:, kt, :], rsb)

            def fc1_piece(i, fs, fe, mdst):
                """fc1+gelu for global ffts [fs, fe) of chunk i into mdst
                (slot = fft % FH)."""
                x2s = x2s_t[i]
                for fft in range(fs, fe):
                    w1t = mp.tile([P, DK, P], BF16, tag="w1t", bufs=3)
                    nc.sync.dma_start(w1t, w1[fft])
                    pp = psum.tile([P, 512], FP32, tag="acc", bufs=2)
                    for kt in range(DK):
                        nc.tensor.matmul(pp, w1t[:, kt, :], x2s[:, kt, :],
                                         start=(kt == 0), stop=(kt == DK - 1))
                    nc.scalar.activation(mdst[:, fft % FH, :], pp,
                                         AF.Gelu_apprx_tanh,
                                         bias=c1_sb[:, fft:fft + 1])

            def fc2_half(i, half, msrc, acc2):
                """fc2 partial over ffts [half*FH, (half+1)*FH)."""
                x2 = x2_sb[:, i]
                for ot in range(DK):
                    w2t = mp.tile([P, FH, P], BF16, tag="w2t", bufs=2)
                    nc.sync.dma_start(w2t, w2[ot, :, half * FH:(half + 1) * FH])
                    pp = psum.tile([P, 512], FP32, tag="acc", bufs=2)
                    for ff in range(FH):
                        nc.tensor.matmul(pp, w2t[:, ff, :],
                                         msrc[:, ff, :],
                                         start=(ff == 0), stop=(ff == FH - 1))
                    if half == 0:
                        # stash (fc2A + b2 + x2) in bf16
                        nc.vector.scalar_tensor_tensor(
                            acc2[:, ot, :], pp, b2_sb[:, ot:ot + 1],
                            x2[:, ot, :], ALU.add, ALU.add)
                    else:
                        ot_t = mp.tile([P, 512], FP32, tag="outt", bufs=1)
                        nc.vector.tensor_tensor(ot_t, pp, acc2[:, ot, :],
                                                ALU.add)
                        nc.sync.dma_start(
                            out[:, ot, i * 512:(i + 1) * 512], ot_t)

            def attn(qt, mlp_cb=None):
                """attention + Wo partial for q-tile qt.  mlp_cb(pt) lets the
                caller interleave MLP pieces between pt bands."""
                nkt = 4 * qt + 4
                qv = qcur_t[qt]
                ysb = mp.tile([P, NOT, 512], BF16, tag="ysb", bufs=2)
                for pt in range(NOT):
                    y_ps = [psum.tile([HD + 1, 512], FP32, name=f"yps{hb}",
                                      tag=f"y{hb}", bufs=1) for hb in range(2)]
                    for kt in range(nkt):
                        ksl = slice(kt * P, (kt + 1) * P)
                        sg = psum.tile([P, 1024], FP32, tag="sg", bufs=2)
                        nc.tensor.matmul(sg[:, 0:512],
                                         kT_sb[0:HD, pt, ksl],
                                         qv[0:HD, pt, :],
                                         start=True, stop=True)
                        nc.tensor.matmul(sg[:, 512:1024],
                                         kT_sb[HD:P, pt, ksl],
                                         qv[HD:P, pt, :],
                                         start=True, stop=True)
                        pe = mp.tile([P, 1024], BF16, tag="pexp", bufs=3)
                        nc.scalar.activation(pe, sg, AF.Exp, scale=SCALE)
                        j = kt - 4 * qt
                        if j >= 0:
                            moff = 384 - P * j
                            nc.vector.tensor_mul(
                                pe[:, 0:512], pe[:, 0:512],
                                masks_sb[:, moff:moff + 512])
                            nc.vector.tensor_mul(
                                pe[:, 512:1024], pe[:, 512:1024],
                                masks_sb[:, moff:moff + 512])
                        nc.tensor.matmul(y_ps[0], v_sb[:, 2 * pt, kt, :],
                                         pe[:, 0:512],
                                         start=(kt == 0), stop=(kt == nkt - 1))
                        nc.tensor.matmul(y_ps[1], v_sb[:, 2 * pt + 1, kt, :],
                                         pe[:, 512:1024],
                                         start=(kt == 0), stop=(kt == nkt - 1))
                    for hb in range(2):
                        den = mp.tile([HD + 1, 512], FP32, tag="den", bufs=1)
                        nc.vector.reciprocal(den[HD:HD + 1, :],
                                             y_ps[hb][HD:HD + 1, :])
                        rec = mp.tile([1, 512], FP32, tag="rec", bufs=1)
                        nc.sync.dma_start(rec, den[HD:HD + 1, :])
                        rb = mp.tile([HD, 512], FP32, tag="rb", bufs=2)
                        nc.gpsimd.partition_broadcast(rb, rec)
                        if hb == 0:
                            nc.vector.tensor_mul(ysb[0:HD, pt, :],
                                                 y_ps[hb][0:HD, :], rb)
                        else:
                            yst = mp.tile([HD, 512], BF16, tag="yst", bufs=1)
                            nc.vector.tensor_mul(yst, y_ps[hb][0:HD, :], rb)
                            nc.sync.dma_start(ysb[HD:P, pt, :], yst)
                    if mlp_cb is not None:
                        mlp_cb(pt)
                # Wo partials
                ast = mp.tile([P, DK, 512], BF16, tag="ast", bufs=1)
                for ot in range(DK):
                    pp = psum.tile([P, 512], FP32, tag="acc", bufs=2)
                    for pt in range(NOT):
                        nc.tensor.matmul(pp, wo_sb[:, pt, ot * P:(ot + 1) * P],
                                         ysb[:, pt, :],
                                         start=(pt == 0), stop=(pt == NOT - 1))
                    nc.vector.tensor_copy(ast[:, ot, :], pp)
                nc.gpsimd.dma_start(
                    rs_in[qt // 2].rearrange("s k p t -> s p k t")[qt % 2],
                    ast)
                if qt % 2 == 1:
                    i = qt // 2
                    nc.gpsimd.collective_compute(
                        "ReduceScatter", ALU.add,
                        replica_groups=[[0, 1], [2, 3], [4, 5], [6, 7]],
                        ins=[rs_in[i].opt()], outs=[rs_out[i].opt()])

            # ==================== program ====================
            # RS pairing: rs_in[0] = {qt0 (slot0), qt1 (slot1)} fires inside
            # attn(1); rs_in[1] = {qt2, qt3} fires inside attn(3).  Core h of
            # each pair owns tokens of q-tiles {h, h+2}, so chunk-0 MLP can
            # overlap attn(2)/attn(3) instead of running after them.
            for tt in range(NT):
                phase_ab(tt)
                if 1 <= tt <= 2:
                    attn(tt - 1)     # attn(1) fires ReduceScatter #0
            m0 = mp.tile([P, FH, 512], BF16, tag="m", bufs=1, name="m0")
            acc2_0 = mp.tile([P, DK, 512], BF16, tag="acc2", bufs=1,
                             name="acc2_0")

            def mlp_a_cb(pt):
                if pt == 2:
                    phase_e(0)
                elif pt == 3:
                    fc1_piece(0, 0, 8, m0)

            def mlp_b_cb(pt):
                if pt == 0:
                    fc1_piece(0, 8, FH, m0)
                elif pt == 1:
                    fc2_half(0, 0, m0, acc2_0)
                elif pt == 2:
                    fc1_piece(0, FH, 24, m0)
                else:
                    fc1_piece(0, 24, FFT, m0)

            attn(2, mlp_cb=mlp_a_cb)
            attn(3, mlp_cb=mlp_b_cb)   # fires ReduceScatter #1
            fc2_half(0, 1, m0, acc2_0)
            phase_e(1)
            m1 = mp.tile([P, FH, 512], BF16, tag="m", bufs=1, name="m1")
            acc2_1 = mp.tile([P, DK, 512], BF16, tag="acc2", bufs=1,
                             name="acc2_1")
            fc1_piece(1, 0, FH, m1)
            fc2_half(1, 0, m1, acc2_1)
            fc1_piece(1, FH, FFT, m1)
            fc2_half(1, 1, m1, acc2_1)

    nc.compile()
    return nc


_NC_CACHE = None


def _get_nc():
    global _NC_CACHE
    if _NC_CACHE is None:
        _NC_CACHE = build_program()
    return _NC_CACHE


def prep_in_maps(x, ln1_g, ln1_b, ln2_g, ln2_b, Wq, bq, Wk, bk, Wv, bv,
                 Wo, bo, W1, b1, W2, b2):
    import ml_dtypes
    f32 = np.float32
    bf16 = ml_dtypes.bfloat16
    x = np.asarray(x, f32)
    ln1_g, ln1_b = np.asarray(ln1_g, f32), np.asarray(ln1_b, f32)
    ln2_g, ln2_b = np.asarray(ln2_g, f32), np.asarray(ln2_b, f32)
    Wq, Wk, Wv, Wo = (np.asarray(a, f32) for a in (Wq, Wk, Wv, Wo))
    W1, W2 = np.asarray(W1, f32), np.asarray(W2, f32)
    bq, bk, bv, bo_, b1, b2_ = (np.asarray(a, f32)
                                for a in (bq, bk, bv, bo, b1, b2))

    # fold LN gain AND the mean subtraction (a rank-1 correction) into W:
    # (x - mu) * g @ W  =  x @ (g*W - colsum(g*W)/D)
    Wqg = ln1_g[:, None] * Wq
    Wkg = ln1_g[:, None] * Wk
    Wvg = ln1_g[:, None] * Wv
    Wqg = Wqg - Wqg.sum(0, keepdims=True) / D
    Wkg = Wkg - Wkg.sum(0, keepdims=True) / D
    Wvg = Wvg - Wvg.sum(0, keepdims=True) / D
    cq_full = ln1_b @ Wq + bq
    ck_full = ln1_b @ Wk + bk
    cv_full = ln1_b @ Wv + bv
    W1g = ln2_g[:, None] * W1
    W1g = W1g - W1g.sum(0, keepdims=True) / D
    c1_full = ln2_b @ W1 + b1

    w1_t = np.ascontiguousarray(
        W1g.reshape(DK, P, FFT, P).transpose(2, 1, 0, 3)).astype(bf16)
    w2_t = np.ascontiguousarray(
        W2.reshape(FFT, P, DK, P).transpose(2, 1, 0, 3)).astype(bf16)
    c1_t = np.ascontiguousarray(c1_full.reshape(FFT, P).T)      # [P,FFT]
    b2_t = np.ascontiguousarray(b2_.reshape(DK, P).T)           # [P,DK]
    bo_t = np.ascontiguousarray(bo_.reshape(DK, P).T)           # [P,DK]

    kk = np.arange(P)[:, None]
    cc = np.arange(896)[None, :]
    mk = (kk + 384 <= cc).astype(bf16)                          # [P,896]

    in_maps = []
    for c in range(8):
        b_idx, hh = c // 2, c % 2
        sl = slice(DQ * hh, DQ * hh + DQ)
        xT_c = np.ascontiguousarray(
            x[b_idx].T.reshape(DK, P, T).transpose(1, 0, 2)).astype(bf16)
        wq_c, wk_c = Wqg[:, sl], Wkg[:, sl]
        wqk_t = np.ascontiguousarray(
            np.stack([wq_c, wk_c]).reshape(2, DK, P, NOT, P)
            .transpose(2, 0, 3, 1, 4)).astype(bf16)             # [P,2,NOT,DK,P]
        wv_t = np.ascontiguousarray(
            Wvg[:, sl].reshape(DK, P, DQ).transpose(1, 0, 2)).astype(bf16)
        wo_t = np.ascontiguousarray(
            Wo[sl, :].reshape(NOT, P, D).transpose(1, 0, 2)).astype(bf16)
        cq_t = cq_full[sl].reshape(NOT, P).T                    # [P,NOT]
        ck_t = ck_full[sl].reshape(NOT, P).T
        own = np.concatenate(
            [xT_c[:, :, hh * 512:(hh + 1) * 512],
             xT_c[:, :, (hh + 2) * 512:(hh + 3) * 512]], axis=2)
        in_maps.append({
            "xT": xT_c,
            "xTo": np.ascontiguousarray(own),
            "wqk": wqk_t,
            "wv": wv_t,
            "wo": wo_t,
            "w1": w1_t,
            "w2": w2_t,
            "cqk": np.ascontiguousarray(np.concatenate([cq_t, ck_t], axis=1)),
            "cvb": np.broadcast_to(
                cv_full[sl][None, :], (P, DQ)).astype(bf16),
            "bo": bo_t,
            "c1": c1_t,
            "b2": b2_t,
            "masks": mk,
        })
    return in_maps


def assemble_output(results):
    out = np.empty((B, T, D), np.float32)
    for c in range(8):
        b_idx, hh = c // 2, c % 2
        o = results[c]["out"]                                   # [P,DK,TOWN]
        full = o.transpose(2, 1, 0).reshape(TOWN, D)
        out[b_idx, hh * 512:(hh + 1) * 512, :] = full[0:512]
        out[b_idx, (hh + 2) * 512:(hh + 3) * 512, :] = full[512:1024]
    return out


def kernel(**inputs):
    nc = _get_nc()
    in_maps = prep_in_maps(**inputs)
    res = run_bass_kernel_spmd(nc, in_maps, list(range(8)))
    return assemble_output(res.results)



# revision 21
# speedup vs baseline: 1.1997x; 1.1118x over previous
"""Trainium2 Bass kernel for a dense transformer block (B=4, T=2048, D=1024, H=16).

Sharding: 8 cores = 4 batches x 2 head-halves.  Each core computes LN1
(mean fold in weights, rstd pre-scaled into the bf16 matmul operand),
Q/K/V for its 8 heads over all 2048 tokens, causal attention with
hb-paired [128,1024] exp groups (denominator via a ones-column in V),
then a row-parallel Wo partial that is pairwise ReduceScattered so each
core ends up with 1024 tokens for LN2 + MLP.  All matmul operands are
bf16 (fp32 PSUM accumulate); weights are SBUF-resident or streamed
exactly once per use.  MLP for the first 512-token chunk is interleaved
into the last attention tile; gelu uses the tanh approximation (stays
near the exp activation table).

Host-side prep (numpy only): fold LayerNorm gains/means into the
projection weights, cast to bf16, lay out every tensor partition-major.
"""

import os
import sys

for _p in ("/opt/trn_rl_repo", "/root/.axon_site/_ro/trn_rl_repo"):
    if os.path.isdir(_p) and _p not in sys.path:
        sys.path.append(_p)

import numpy as np

import concourse.bass as bass
import concourse.tile as tile
from concourse import bacc, mybir
from concourse.bass_utils import run_bass_kernel_spmd

AF = mybir.ActivationFunctionType
ALU = mybir.AluOpType
FP32 = mybir.dt.float32
FP32R = mybir.dt.float32r
BF16 = mybir.dt.bfloat16

B, T, D, H = 4, 2048, 1024, 16
HD = D // H          # 64
DFF = 4 * D          # 4096
P = 128
DK = D // P          # 8   D k-tiles
NT = T // 512        # 4   512-token tiles
HC = H // 2          # 8   local heads per core
DQ = HC * HD         # 512 local qkv width
NOT = DQ // P        # 4   local qkv feature tiles (head pairs)
FFT = DFF // P       # 32  DFF tiles
FH = FFT // 2        # 16  half-FFT group
TOWN = T // 2        # 1024 own tokens after ReduceScatter
EPS = 1e-5
SCALE = 1.0 / 8.0    # 1/sqrt(HD)


def build_program():
    nc = bacc.Bacc("TRN2", target_bir_lowering=False, debug=False)

    # ---- DRAM I/O (all partition-major contiguous) ----
    xT = nc.dram_tensor("xT", [P, DK, T], BF16, kind="ExternalInput")
    xTo = nc.dram_tensor("xTo", [P, DK, TOWN], BF16, kind="ExternalInput")
    wqk = nc.dram_tensor("wqk", [P, 2, NOT, DK, P], BF16, kind="ExternalInput")
    wv = nc.dram_tensor("wv", [P, DK, DQ], BF16, kind="ExternalInput")
    wo = nc.dram_tensor("wo", [P, NOT, D], BF16, kind="ExternalInput")
    w1 = nc.dram_tensor("w1", [FFT, P, DK, P], BF16, kind="ExternalInput")
    w2 = nc.dram_tensor("w2", [DK, P, FFT, P], BF16, kind="ExternalInput")
    cqk = nc.dram_tensor("cqk", [P, 2 * NOT], FP32, kind="ExternalInput")
    cvb = nc.dram_tensor("cvb", [P, DQ], BF16, kind="ExternalInput")
    bo = nc.dram_tensor("bo", [P, DK], FP32, kind="ExternalInput")
    c1 = nc.dram_tensor("c1", [P, FFT], FP32, kind="ExternalInput")
    b2 = nc.dram_tensor("b2", [P, DK], FP32, kind="ExternalInput")
    masks = nc.dram_tensor("masks", [P, 896], BF16, kind="ExternalInput")
    out = nc.dram_tensor("out", [P, DK, TOWN], FP32, kind="ExternalOutput")

    with tile.TileContext(nc) as tc:
        with (
            tc.tile_pool(name="mp", bufs=1) as mp,
            tc.tile_pool(name="psum", bufs=1, space="PSUM") as psum,
            tc.tile_pool(name="dram", bufs=1, space="DRAM") as dram,
        ):
            # ---- persistent small tiles ----
            ones_b = mp.tile([P, 1], BF16)
            nc.vector.memset(ones_b, 1.0)
            ones128 = mp.tile([P, P], BF16)
            nc.vector.memset(ones128, 1.0)
            eps_sb = mp.tile([P, 1], FP32)
            nc.vector.memset(eps_sb, EPS)
            # prefetch first x tile ahead of the big weight loads
            xt0 = mp.tile([P, DK, 512], BF16, tag="xtrot", bufs=2, name="xt0")
            nc.sync.dma_start(xt0, xT[:, :, 0:512])
            # ---- resident weights (wqk first: needed ~10us in) ----
            wqk_sb = mp.tile([P, 2, NOT, DK, P], BF16)
            nc.sync.dma_start(wqk_sb, wqk[:, :, :, :, :])
            wv_sb = mp.tile([P, DK, DQ], BF16)
            nc.sync.dma_start(wv_sb, wv[:, :, :])
            cqk_sb = mp.tile([P, 2 * NOT], FP32)
            nc.sync.dma_start(cqk_sb, cqk[:, :])
            cvb_sb = mp.tile([P, DQ], BF16)
            nc.sync.dma_start(cvb_sb, cvb[:, :])
            wo_sb = mp.tile([P, NOT, D], BF16)
            nc.sync.dma_start(wo_sb, wo[:, :, :])
            bo_sb = mp.tile([P, DK], FP32)
            nc.sync.dma_start(bo_sb, bo[:, :])
            c1_sb = mp.tile([P, FFT], FP32)
            nc.sync.dma_start(c1_sb, c1[:, :])
            b2_sb = mp.tile([P, DK], FP32)
            nc.sync.dma_start(b2_sb, b2[:, :])
            masks_sb = mp.tile([P, 896], BF16)
            nc.sync.dma_start(masks_sb, masks[:, :])

            # ---- resident activations ----
            kT_sb = mp.tile([P, NOT, T], BF16)
            v_sb = mp.tile([P, HC, T // P, HD + 1], BF16)
            for h in range(HC):
                nc.vector.memset(v_sb[:, h, :, HD:HD + 1], 1.0)

            # DRAM scratch for the pairwise ReduceScatter
            rs_in = [dram.tile([2, DK, P, 512], BF16, name=f"rsin{i}",
                               tag=f"rsin{i}") for i in range(2)]
            rs_out = [dram.tile([DK, P, 512], BF16, name=f"rsout{i}",
                                tag=f"rsout{i}") for i in range(2)]

            qcur_t = [None] * NT
            x2s_t = [None] * 2
            x2_t = [None] * 2

            def ln_stats(src3, nblk):
                """rstd of columns of src3 ([P, nblk, 512] bf16) -> sbuf
                [P,512] fp32, already broadcast across partitions (the sum
                matmuls use a ones [P,P] stationary so every partition of the
                PSUM result holds the column sum)."""
                s_ps = psum.tile([P, 512], FP32, tag="acc", bufs=2)
                for kt in range(nblk):
                    nc.tensor.matmul(s_ps, ones128, src3[:, kt, :],
                                     start=(kt == 0), stop=(kt == nblk - 1))
                q_ps = psum.tile([P, 512], FP32, tag="acc", bufs=2)
                for kt in range(nblk):
                    xsq = mp.tile([P, 512], BF16, tag="xsq", bufs=1)
                    eng = nc.vector if kt % 2 == 0 else nc.gpsimd
                    eng.tensor_mul(xsq, src3[:, kt, :], src3[:, kt, :])
                    nc.tensor.matmul(q_ps, ones128, xsq,
                                     start=(kt == 0), stop=(kt == nblk - 1))
                trow = mp.tile([P, 512], FP32, tag="trow", bufs=1)
                vrow = mp.tile([P, 512], FP32, tag="vrow", bufs=1)
                nc.vector.tensor_scalar(trow, s_ps, 1.0 / D, None, ALU.mult)
                nc.vector.tensor_mul(trow, trow, trow)
                nc.vector.scalar_tensor_tensor(vrow, q_ps, 1.0 / D, trow,
                                               ALU.mult, ALU.subtract)
                nc.scalar.activation(vrow, vrow, AF.Sqrt, bias=eps_sb)
                rsb = mp.tile([P, 512], FP32, tag="rsb", bufs=2)
                nc.vector.reciprocal(rsb, vrow)
                return rsb

            def phase_ab(tt):
                ts5 = slice(tt * 512, (tt + 1) * 512)
                if tt == 0:
                    xt = xt0
                else:
                    xt = mp.tile([P, DK, 512], BF16, tag="xtrot", bufs=2)
                    nc.sync.dma_start(xt, xT[:, :, ts5])
                rsb = ln_stats(xt, DK)
                # pre-scaled LN1 output (bf16 matmul operand)
                h_sb = mp.tile([P, DK, 512], BF16, tag="h", bufs=1)
                for kt in range(DK):
                    eng = nc.vector if kt % 2 == 0 else nc.gpsimd
                    eng.tensor_mul(h_sb[:, kt, :], xt[:, kt, :], rsb)
                # q/k projections
                qcur = mp.tile([P, NOT, 512], BF16, tag="qcur", bufs=2)
                qcur_t[tt] = qcur
                for proj in range(2):
                    for ot in range(NOT):
                        pp = psum.tile([P, 512], FP32, tag="acc", bufs=2)
                        for kt in range(DK):
                            nc.tensor.matmul(pp, wqk_sb[:, proj, ot, kt, :],
                                             h_sb[:, kt, :],
                                             start=(kt == 0),
                                             stop=(kt == DK - 1))
                        dest = (qcur[:, ot, :] if proj == 0
                                else kT_sb[:, ot, ts5])
                        nc.vector.tensor_scalar(
                            dest, pp,
                            cqk_sb[:, proj * NOT + ot:proj * NOT + ot + 1],
                            None, ALU.add)
                # v projection (token-major)
                for st in range(4):
                    pp = psum.tile([P, 512], FP32, tag="acc", bufs=2)
                    for kt in range(DK):
                        nc.tensor.matmul(
                            pp, h_sb[:, kt, st * P:(st + 1) * P],
                            wv_sb[:, kt, :],
                            start=(kt == 0), stop=(kt == DK - 1))
                    nc.vector.tensor_tensor(
                        v_sb[:, :, tt * 4 + st, 0:HD],
                        pp.rearrange("p (h e) -> p h e", h=HC),
                        cvb_sb.rearrange("p (h e) -> p h e", h=HC), ALU.add)

            def phase_e(i):
                """x2 = x + Wo-out + bo for 512-token chunk i, then LN2."""
                x2 = mp.tile([P, DK, 512], BF16, tag="x2", bufs=1)
                x2_t[i] = x2
                nc.gpsimd.dma_start(x2, rs_out[i].rearrange("k p t -> p k t"))
                xo = mp.tile([P, DK, 512], BF16, tag="xtrot", bufs=2)
                nc.sync.dma_start(xo, xTo[:, :, i * 512:(i + 1) * 512])
                for kt in range(DK):
                    nc.vector.scalar_tensor_tensor(
                        x2[:, kt, :], x2[:, kt, :], bo_sb[:, kt:kt + 1],
                        xo[:, kt, :], ALU.add, ALU.add)
                rsb = ln_stats(x2, DK)
                x2s = mp.tile([P, DK, 512], BF16, tag="x2s", bufs=1)
                x2s_t[i] = x2s
                for kt in range(DK):
                    eng = nc.vector if kt % 2 == 0 else nc.gpsimd
                    eng.tensor_mul(x2s[:, kt, :], x2[:, kt, :], rsb)

            def fc1_piece(i, fs, fe, mdst):
                """fc1+gelu for global ffts [fs, fe) of chunk i into mdst
                (slot = fft % FH)."""
                x2s = x2s_t[i]
                for fft in range(fs, fe):
                    w1t = mp.tile([P, DK, P], BF16, tag="w1t", bufs=3)
                    nc.sync.dma_start(w1t, w1[fft])
                    pp = psum.tile([P, 512], FP32, tag="acc", bufs=2)
                    for kt in range(DK):
                        nc.tensor.matmul(pp, w1t[:, kt, :], x2s[:, kt, :],
                                         start=(kt == 0), stop=(kt == DK - 1))
                    nc.scalar.activation(mdst[:, fft % FH, :], pp,
                                         AF.Gelu_apprx_tanh,
                                         bias=c1_sb[:, fft:fft + 1])

            def fc2_half(i, half, msrc, acc2):
                """fc2 partial over ffts [half*FH, (half+1)*FH)."""
                x2 = x2_t[i]
                for ot in range(DK):
                    w2t = mp.tile([P, FH, P], BF16, tag="w2t", bufs=2)
                    nc.sync.dma_start(w2t, w2[ot, :, half * FH:(half + 1) * FH])
                    pp = psum.tile([P, 512], FP32, tag="acc", bufs=2)
                    for ff in range(FH):
                        nc.tensor.matmul(pp, w2t[:, ff, :],
                                         msrc[:, ff, :],
                                         start=(ff == 0), stop=(ff == FH - 1))
                    if half == 0:
                        # stash (fc2A + b2 + x2) in bf16
                        nc.vector.scalar_tensor_tensor(
                            acc2[:, ot, :], pp, b2_sb[:, ot:ot + 1],
                            x2[:, ot, :], ALU.add, ALU.add)
                    else:
                        ot_t = mp.tile([P, 512], FP32, tag="outt", bufs=1)
                        nc.vector.tensor_tensor(ot_t, pp, acc2[:, ot, :],
                                                ALU.add)
                        nc.sync.dma_start(
                            out[:, ot, i * 512:(i + 1) * 512], ot_t)

            def attn(qt, mlp_cb=None):
                """attention + Wo partial for q-tile qt.  mlp_cb(pt) lets the
                caller interleave MLP pieces between pt bands.  Softmax
                normalization is deferred: y and the denominator row are
                copied out of PSUM per head (bf16), then one batched
                reciprocal [8,512] per q-tile normalizes everything."""
                nkt = 4 * qt + 4
                qv = qcur_t[qt]
                ysb = mp.tile([P, NOT, 512], BF16, tag="ysb", bufs=2)
                ycop = mp.tile([HD + 1, 2 * NOT, 512], BF16, tag="ycop",
                               bufs=1)
                dent = mp.tile([2 * NOT, 512], BF16, tag="dent", bufs=1)
                for pt in range(NOT):
                    y_ps = [psum.tile([HD + 1, 512], FP32, name=f"yps{hb}",
                                      tag=f"y{hb}", bufs=1) for hb in range(2)]
                    for kt in range(nkt):
                        ksl = slice(kt * P, (kt + 1) * P)
                        sg = psum.tile([P, 1024], FP32, tag="sg", bufs=2)
                        nc.tensor.matmul(sg[:, 0:512],
                                         kT_sb[0:HD, pt, ksl],
                                         qv[0:HD, pt, :],
                                         start=True, stop=True)
                        nc.tensor.matmul(sg[:, 512:1024],
                                         kT_sb[HD:P, pt, ksl],
                                         qv[HD:P, pt, :],
                                         start=True, stop=True)
                        pe = mp.tile([P, 1024], BF16, tag="pexp", bufs=3)
                        nc.scalar.activation(pe, sg, AF.Exp, scale=SCALE)
                        j = kt - 4 * qt
                        if j >= 0:
                            moff = 384 - P * j
                            nc.vector.tensor_mul(
                                pe[:, 0:512], pe[:, 0:512],
                                masks_sb[:, moff:moff + 512])
                            nc.vector.tensor_mul(
                                pe[:, 512:1024], pe[:, 512:1024],
                                masks_sb[:, moff:moff + 512])
                        nc.tensor.matmul(y_ps[0], v_sb[:, 2 * pt, kt, :],
                                         pe[:, 0:512],
                                         start=(kt == 0), stop=(kt == nkt - 1))
                        nc.tensor.matmul(y_ps[1], v_sb[:, 2 * pt + 1, kt, :],
                                         pe[:, 512:1024],
                                         start=(kt == 0), stop=(kt == nkt - 1))
                    for hb in range(2):
                        j = 2 * pt + hb
                        nc.vector.tensor_copy(ycop[:, j, :], y_ps[hb])
                        nc.sync.dma_start(dent[j:j + 1, :],
                                          ycop[HD:HD + 1, j, :])
                    if mlp_cb is not None:
                        mlp_cb(pt)
                # batched softmax denominators for all 8 heads of this q-tile
                rcp = mp.tile([2 * NOT, 512], FP32, tag="rcp", bufs=1)
                nc.vector.reciprocal(rcp, dent)
                for pt in range(NOT):
                    for hb in range(2):
                        j = 2 * pt + hb
                        rec = mp.tile([1, 512], FP32, tag="rec", bufs=1)
                        nc.sync.dma_start(rec, rcp[j:j + 1, :])
                        rb = mp.tile([HD, 512], FP32, tag="rb", bufs=2)
                        nc.gpsimd.partition_broadcast(rb, rec)
                        if hb == 0:
                            nc.vector.tensor_mul(ysb[0:HD, pt, :],
                                                 ycop[0:HD, j, :], rb)
                        else:
                            yst = mp.tile([HD, 512], BF16, tag="yst", bufs=1)
                            nc.vector.tensor_mul(yst, ycop[0:HD, j, :], rb)
                            nc.sync.dma_start(ysb[HD:P, pt, :], yst)
                # Wo partials
                ast = mp.tile([P, DK, 512], BF16, tag="ast", bufs=1)
                for ot in range(DK):
                    pp = psum.tile([P, 512], FP32, tag="acc", bufs=2)
                    for pt in range(NOT):
                        nc.tensor.matmul(pp, wo_sb[:, pt, ot * P:(ot + 1) * P],
                                         ysb[:, pt, :],
                                         start=(pt == 0), stop=(pt == NOT - 1))
                    nc.vector.tensor_copy(ast[:, ot, :], pp)
                nc.gpsimd.dma_start(
                    rs_in[qt // 2].rearrange("s k p t -> s p k t")[qt % 2],
                    ast)
                if qt % 2 == 1:
                    i = qt // 2
                    nc.gpsimd.collective_compute(
                        "ReduceScatter", ALU.add,
                        replica_groups=[[0, 1], [2, 3], [4, 5], [6, 7]],
                        ins=[rs_in[i].opt()], outs=[rs_out[i].opt()])

            # ==================== program ====================
            # RS pairing: rs_in[0] = {qt0 (slot0), qt1 (slot1)} fires inside
            # attn(1); rs_in[1] = {qt2, qt3} fires inside attn(3).  Core h of
            # each pair owns tokens of q-tiles {h, h+2}, so chunk-0 MLP can
            # overlap attn(2)/attn(3) instead of running after them.
            for tt in range(NT):
                phase_ab(tt)
                if 1 <= tt <= 2:
                    attn(tt - 1)     # attn(1) fires ReduceScatter #0
            m0 = mp.tile([P, FH, 512], BF16, tag="m", bufs=1, name="m0")
            acc2_0 = mp.tile([P, DK, 512], BF16, tag="acc2", bufs=1,
                             name="acc2_0")

            def mlp_a_cb(pt):
                if pt == 2:
                    phase_e(0)
                elif pt == 3:
                    fc1_piece(0, 0, 8, m0)

            def mlp_b_cb(pt):
                if pt == 0:
                    fc1_piece(0, 8, FH, m0)
                elif pt == 1:
                    fc2_half(0, 0, m0, acc2_0)
                elif pt == 2:
                    fc1_piece(0, FH, 24, m0)
                else:
                    fc1_piece(0, 24, FFT, m0)

            attn(2, mlp_cb=mlp_a_cb)
            attn(3, mlp_cb=mlp_b_cb)   # fires ReduceScatter #1
            fc2_half(0, 1, m0, acc2_0)
            phase_e(1)
            m1 = mp.tile([P, FH, 512], BF16, tag="m", bufs=1, name="m1")
            acc2_1 = mp.tile([P, DK, 512], BF16, tag="acc2", bufs=1,
                             name="acc2_1")
            fc1_piece(1, 0, FH, m1)
            fc2_half(1, 0, m1, acc2_1)
            fc1_piece(1, FH, FFT, m1)
            fc2_half(1, 1, m1, acc2_1)

    nc.compile()
    return nc


_NC_CACHE = None


def _get_nc():
    global _NC_CACHE
    if _NC_CACHE is None:
        _NC_CACHE = build_program()
    return _NC_CACHE


def prep_in_maps(x, ln1_g, ln1_b, ln2_g, ln2_b, Wq, bq, Wk, bk, Wv, bv,
                 Wo, bo, W1, b1, W2, b2):
    import ml_dtypes
    f32 = np.float32
    bf16 = ml_dtypes.bfloat16
    x = np.asarray(x, f32)
    ln1_g, ln1_b = np.asarray(ln1_g, f32), np.asarray(ln1_b, f32)
    ln2_g, ln2_b = np.asarray(ln2_g, f32), np.asarray(ln2_b, f32)
    Wq, Wk, Wv, Wo = (np.asarray(a, f32) for a in (Wq, Wk, Wv, Wo))
    W1, W2 = np.asarray(W1, f32), np.asarray(W2, f32)
    bq, bk, bv, bo_, b1, b2_ = (np.asarray(a, f32)
                                for a in (bq, bk, bv, bo, b1, b2))

    # fold LN gain AND the mean subtraction (a rank-1 correction) into W:
    # (x - mu) * g @ W  =  x @ (g*W - colsum(g*W)/D)
    Wqg = ln1_g[:, None] * Wq
    Wkg = ln1_g[:, None] * Wk
    Wvg = ln1_g[:, None] * Wv
    Wqg = Wqg - Wqg.sum(0, keepdims=True) / D
    Wkg = Wkg - Wkg.sum(0, keepdims=True) / D
    Wvg = Wvg - Wvg.sum(0, keepdims=True) / D
    cq_full = ln1_b @ Wq + bq
    ck_full = ln1_b @ Wk + bk
    cv_full = ln1_b @ Wv + bv
    W1g = ln2_g[:, None] * W1
    W1g = W1g - W1g.sum(0, keepdims=True) / D
    c1_full = ln2_b @ W1 + b1

    w1_t = np.ascontiguousarray(
        W1g.reshape(DK, P, FFT, P).transpose(2, 1, 0, 3)).astype(bf16)
    w2_t = np.ascontiguousarray(
        W2.reshape(FFT, P, DK, P).transpose(2, 1, 0, 3)).astype(bf16)
    c1_t = np.ascontiguousarray(c1_full.reshape(FFT, P).T)      # [P,FFT]
    b2_t = np.ascontiguousarray(b2_.reshape(DK, P).T)           # [P,DK]
    bo_t = np.ascontiguousarray(bo_.reshape(DK, P).T)           # [P,DK]

    kk = np.arange(P)[:, None]
    cc = np.arange(896)[None, :]
    mk = (kk + 384 <= cc).astype(bf16)                          # [P,896]

    in_maps = []
    for c in range(8):
        b_idx, hh = c // 2, c % 2
        sl = slice(DQ * hh, DQ * hh + DQ)
        xT_c = np.ascontiguousarray(
            x[b_idx].T.reshape(DK, P, T).transpose(1, 0, 2)).astype(bf16)
        wq_c, wk_c = Wqg[:, sl], Wkg[:, sl]
        wqk_t = np.ascontiguousarray(
            np.stack([wq_c, wk_c]).reshape(2, DK, P, NOT, P)
            .transpose(2, 0, 3, 1, 4)).astype(bf16)             # [P,2,NOT,DK,P]
        wv_t = np.ascontiguousarray(
            Wvg[:, sl].reshape(DK, P, DQ).transpose(1, 0, 2)).astype(bf16)
        wo_t = np.ascontiguousarray(
            Wo[sl, :].reshape(NOT, P, D).transpose(1, 0, 2)).astype(bf16)
        cq_t = cq_full[sl].reshape(NOT, P).T                    # [P,NOT]
        ck_t = ck_full[sl].reshape(NOT, P).T
        own = np.concatenate(
            [xT_c[:, :, hh * 512:(hh + 1) * 512],
             xT_c[:, :, (hh + 2) * 512:(hh + 3) * 512]], axis=2)
        in_maps.append({
            "xT": xT_c,
            "xTo": np.ascontiguousarray(own),
            "wqk": wqk_t,
            "wv": wv_t,
            "wo": wo_t,
            "w1": w1_t,
            "w2": w2_t,
            "cqk": np.ascontiguousarray(np.concatenate([cq_t, ck_t], axis=1)),
            "cvb": np.broadcast_to(
                cv_full[sl][None, :], (P, DQ)).astype(bf16),
            "bo": bo_t,
            "c1": c1_t,
            "b2": b2_t,
            "masks": mk,
        })
    return in_maps


def assemble_output(results):
    out = np.empty((B, T, D), np.float32)
    for c in range(8):
        b_idx, hh = c // 2, c % 2
        o = results[c]["out"]                                   # [P,DK,TOWN]
        full = o.transpose(2, 1, 0).reshape(TOWN, D)
        out[b_idx, hh * 512:(hh + 1) * 512, :] = full[0:512]
        out[b_idx, (hh + 2) * 512:(hh + 3) * 512, :] = full[512:1024]
    return out


def kernel(**inputs):
    nc = _get_nc()
    in_maps = prep_in_maps(**inputs)
    res = run_bass_kernel_spmd(nc, in_maps, list(range(8)))
    return assemble_output(res.results)



# revision 53
# speedup vs baseline: 1.2599x; 1.0501x over previous
"""Trainium2 Bass kernel for a dense transformer block (B=4, T=2048, D=1024, H=16).

Sharding: 8 cores = 4 batches x 2 head-halves.  Each core computes LN1
(mean fold in weights, rstd pre-scaled into the bf16 matmul operand),
Q/K/V for its 8 heads over all 2048 tokens, causal attention with
hb-paired [128,1024] exp groups (denominator via a ones-column in V),
then a row-parallel Wo partial that is pairwise ReduceScattered so each
core ends up with 1024 tokens for LN2 + MLP.

Schedule/engine notes (from perfetto/NTFF iteration):
- RS groups are {qt0,qt1} / {qt2,qt3} with core h owning q-tiles
  {h, h+2}: chunk-0 MLP overlaps attention tiles 2-3 instead of
  running after them; RS buffers are [P,DK,512]-contiguous so both
  sides are static DMAs (a transposing dynamic DMA on gpsimd blocked
  the queue for the whole collective latency).
- LN sums use a ones[128,128] stationary so the PSUM result is already
  partition-broadcast (no gpsimd broadcast hop); each tile's LN stats
  are issued one tile ahead so the reciprocal chain hides under the
  previous tile's projections; rstd is cast to bf16 so the h muls hit
  the 4x bf16 DVE mode.
- Softmax normalization is deferred: y+denominator rows are copied out
  of PSUM per head, one batched reciprocal [8,512] per q-tile (the DVE
  RECIPROCAL is ~3.3us regardless of partition count).
- Chunk-0 fc1 writes raw pre-activations; gelu runs in batches so the
  ACT table isn't thrashed between EXP and GELU (1.3us per switch).
- Post-ReduceScatter elementwise work runs on gpsimd and phase_e(0) is
  floored with tile_wait_until so the scheduler (which underestimates
  collective latency) cannot hoist it into an engine queue where its
  semaphore wait would stall independent attention work.
- The two m (fc1 activation) tiles share a 2-slot pool with wqk, which
  dies exactly when the first m tile is written: double-buffered MLP
  halves at zero extra SBUF.

Host-side prep (numpy only): fold LayerNorm gains/means into the
projection weights (and bo into xTo), cast to bf16, lay out every
tensor partition-major.
"""

import os
import sys

for _p in ("/opt/trn_rl_repo", "/root/.axon_site/_ro/trn_rl_repo"):
    if os.path.isdir(_p) and _p not in sys.path:
        sys.path.append(_p)

import numpy as np

import concourse.bass as bass
import concourse.tile as tile
from concourse import bacc, mybir
from concourse.bass_utils import run_bass_kernel_spmd

AF = mybir.ActivationFunctionType
ALU = mybir.AluOpType
FP32 = mybir.dt.float32
FP32R = mybir.dt.float32r
BF16 = mybir.dt.bfloat16

B, T, D, H = 4, 2048, 1024, 16
HD = D // H          # 64
DFF = 4 * D          # 4096
P = 128
DK = D // P          # 8   D k-tiles
NT = T // 512        # 4   512-token tiles
HC = H // 2          # 8   local heads per core
DQ = HC * HD         # 512 local qkv width
NOT = DQ // P        # 4   local qkv feature tiles (head pairs)
FFT = DFF // P       # 32  DFF tiles
FH = FFT // 2        # 16  half-FFT group
TOWN = T // 2        # 1024 own tokens after ReduceScatter
EPS = 1e-5
SCALE = 1.0 / 8.0    # 1/sqrt(HD)


def build_program():
    nc = bacc.Bacc("TRN2", target_bir_lowering=False, debug=False)

    # ---- DRAM I/O (all partition-major contiguous) ----
    xT = nc.dram_tensor("xT", [P, DK, T], BF16, kind="ExternalInput")
    xTo = nc.dram_tensor("xTo", [P, DK, TOWN], BF16, kind="ExternalInput")
    wqk = nc.dram_tensor("wqk", [P, 2, NOT, DK, P], BF16, kind="ExternalInput")
    wv = nc.dram_tensor("wv", [P, DK, DQ], BF16, kind="ExternalInput")
    wo = nc.dram_tensor("wo", [P, NOT, D], BF16, kind="ExternalInput")
    w1 = nc.dram_tensor("w1", [FFT, P, DK, P], BF16, kind="ExternalInput")
    w2 = nc.dram_tensor("w2", [DK, P, FFT, P], BF16, kind="ExternalInput")
    cqk = nc.dram_tensor("cqk", [P, 2 * NOT], FP32, kind="ExternalInput")
    cvb = nc.dram_tensor("cvb", [P, DQ], BF16, kind="ExternalInput")
    c1 = nc.dram_tensor("c1", [P, FFT], FP32, kind="ExternalInput")
    b2 = nc.dram_tensor("b2", [P, DK], FP32, kind="ExternalInput")
    masks = nc.dram_tensor("masks", [P, 896], BF16, kind="ExternalInput")
    out = nc.dram_tensor("out", [P, DK, TOWN], FP32, kind="ExternalOutput")

    with tile.TileContext(nc) as tc:
        with (
            tc.tile_pool(name="mp", bufs=1) as mp,
            tc.tile_pool(name="psum", bufs=1, space="PSUM") as psum,
            tc.tile_pool(name="dram", bufs=1, space="DRAM") as dram,
        ):
            # ---- persistent small tiles ----
            ones_b = mp.tile([P, 1], BF16)
            nc.vector.memset(ones_b, 1.0)
            ones128 = mp.tile([P, P], BF16)
            nc.vector.memset(ones128, 1.0)
            eps_sb = mp.tile([P, 1], FP32)
            nc.vector.memset(eps_sb, EPS)
            # prefetch first x tile ahead of the big weight loads
            xt0 = mp.tile([P, DK, 512], BF16, tag="xtrot", bufs=2, name="xt0")
            nc.sync.dma_start(xt0, xT[:, :, 0:512])
            # ---- resident weights (wqk first: needed ~10us in) ----
            # wqk shares its slot pool with the MLP m tiles: it dies after
            # phase_ab(3), exactly when the first m tile is written, so the
            # two double-buffered m tiles cost no extra SBUF.
            wqk_sb = mp.tile([P, 2, NOT, DK, P], BF16, tag="bigbuf", bufs=2,
                             name="wqk_sb")
            nc.sync.dma_start(wqk_sb, wqk[:, :, :, :, :])
            wv_sb = mp.tile([P, DK, DQ], BF16)
            nc.sync.dma_start(wv_sb, wv[:, :, :])
            cqk_sb = mp.tile([P, 2 * NOT], FP32)
            nc.sync.dma_start(cqk_sb, cqk[:, :])
            cvb_sb = mp.tile([P, DQ], BF16)
            nc.sync.dma_start(cvb_sb, cvb[:, :])
            wo_sb = mp.tile([P, NOT, D], BF16)
            nc.sync.dma_start(wo_sb, wo[:, :, :])
            c1_sb = mp.tile([P, FFT], FP32)
            nc.sync.dma_start(c1_sb, c1[:, :])
            b2_sb = mp.tile([P, DK], FP32)
            nc.sync.dma_start(b2_sb, b2[:, :])
            masks_sb = mp.tile([P, 896], BF16)
            nc.sync.dma_start(masks_sb, masks[:, :])

            # ---- resident activations ----
            kT_sb = mp.tile([P, NOT, T], BF16)
            v_sb = mp.tile([P, HC, T // P, HD + 1], BF16)
            for h in range(HC):
                nc.vector.memset(v_sb[:, h, :, HD:HD + 1], 1.0)

            # DRAM scratch for the pairwise ReduceScatter.  [P, DK, 512]
            # slot layout so both the producer write and the consumer read
            # are contiguous static DMAs (no engine-blocking dynamic DMA).
            rs_in = [dram.tile([2, P, DK, 512], BF16, name=f"rsin{i}",
                               tag=f"rsin{i}") for i in range(2)]
            rs_out = [dram.tile([P, DK, 512], BF16, name=f"rsout{i}",
                                tag=f"rsout{i}") for i in range(2)]

            qcur_t = [None] * NT
            x2s_t = [None] * 2
            x2_t = [None] * 2

            def ln_stats(src3, nblk, sq_eng=None):
                """rstd of columns of src3 ([P, nblk, 512] bf16) -> sbuf
                [P,512] bf16, already broadcast across partitions (the sum
                matmuls use a ones [P,P] stationary so every partition of the
                PSUM result holds the column sum).  sq_eng picks the engine
                for the x^2 muls: gpsimd for the post-ReduceScatter LN2 so a
                stalled dependency never blocks the DVE queue."""
                sq = nc.vector if sq_eng is None else sq_eng
                s_ps = psum.tile([P, 512], FP32, tag="acc", bufs=2)
                for kt in range(nblk):
                    nc.tensor.matmul(s_ps, ones128, src3[:, kt, :],
                                     start=(kt == 0), stop=(kt == nblk - 1))
                q_ps = psum.tile([P, 512], FP32, tag="acc", bufs=2)
                for kt in range(nblk):
                    xsq = mp.tile([P, 512], BF16, tag="xsq", bufs=1)
                    sq.tensor_mul(xsq, src3[:, kt, :], src3[:, kt, :])
                    nc.tensor.matmul(q_ps, ones128, xsq,
                                     start=(kt == 0), stop=(kt == nblk - 1))
                trow = mp.tile([P, 512], FP32, tag="trow", bufs=1)
                vrow = mp.tile([P, 512], FP32, tag="vrow", bufs=1)
                nc.vector.tensor_scalar(trow, s_ps, 1.0 / D, None, ALU.mult)
                nc.vector.tensor_mul(trow, trow, trow)
                nc.vector.scalar_tensor_tensor(vrow, q_ps, 1.0 / D, trow,
                                               ALU.mult, ALU.subtract)
                nc.scalar.activation(vrow, vrow, AF.Sqrt, bias=eps_sb)
                rsf = mp.tile([P, 512], FP32, tag="rsf", bufs=1)
                nc.vector.reciprocal(rsf, vrow)
                # bf16 rstd -> the h muls hit the 4x bf16 DVE mode
                rsb = mp.tile([P, 512], BF16, tag="rsb", bufs=2)
                nc.vector.tensor_copy(rsb, rsf)
                return rsb

            def ln_part(tt):
                """load x tile tt + LN1 stats; issued one tile ahead so the
                rstd chain hides under the previous tile's projections."""
                if tt == 0:
                    xt = xt0
                else:
                    xt = mp.tile([P, DK, 512], BF16, tag="xtrot", bufs=2)
                    nc.sync.dma_start(xt, xT[:, :, tt * 512:(tt + 1) * 512])
                return xt, ln_stats(xt, DK)

            def phase_ab(tt, xt, rsb):
                ts5 = slice(tt * 512, (tt + 1) * 512)
                # pre-scaled LN1 output (bf16 matmul operand)
                h_sb = mp.tile([P, DK, 512], BF16, tag="h", bufs=1)
                for kt in range(DK):
                    eng = nc.vector if kt % 2 == 0 else nc.gpsimd
                    eng.tensor_mul(h_sb[:, kt, :], xt[:, kt, :], rsb)
                # q/k projections
                qcur = mp.tile([P, NOT, 512], BF16, tag="qcur", bufs=2)
                qcur_t[tt] = qcur
                for proj in range(2):
                    for ot in range(NOT):
                        pp = psum.tile([P, 512], FP32, tag="acc", bufs=2)
                        for kt in range(DK):
                            nc.tensor.matmul(pp, wqk_sb[:, proj, ot, kt, :],
                                             h_sb[:, kt, :],
                                             start=(kt == 0),
                                             stop=(kt == DK - 1))
                        dest = (qcur[:, ot, :] if proj == 0
                                else kT_sb[:, ot, ts5])
                        nc.vector.tensor_scalar(
                            dest, pp,
                            cqk_sb[:, proj * NOT + ot:proj * NOT + ot + 1],
                            None, ALU.add)
                # v projection (token-major)
                for st in range(4):
                    pp = psum.tile([P, 512], FP32, tag="acc", bufs=2)
                    for kt in range(DK):
                        nc.tensor.matmul(
                            pp, h_sb[:, kt, st * P:(st + 1) * P],
                            wv_sb[:, kt, :],
                            start=(kt == 0), stop=(kt == DK - 1))
                    nc.vector.tensor_tensor(
                        v_sb[:, :, tt * 4 + st, 0:HD],
                        pp.rearrange("p (h e) -> p h e", h=HC),
                        cvb_sb.rearrange("p (h e) -> p h e", h=HC), ALU.add)

            def phase_e(i):
                """x2 = x + Wo-out + bo for 512-token chunk i, then LN2."""
                x2 = mp.tile([P, DK, 512], BF16, tag="x2", bufs=1)
                x2_t[i] = x2
                nc.sync.dma_start(x2, rs_out[i])
                xo = mp.tile([P, DK, 512], BF16, tag="xtrot", bufs=2)
                nc.sync.dma_start(xo, xTo[:, :, i * 512:(i + 1) * 512])
                # gpsimd: these wait on the ReduceScatter; keeping them off
                # the DVE queue stops the wait from stalling attention work.
                # bo is folded into xTo host-side.
                for kt in range(DK):
                    nc.gpsimd.tensor_tensor(
                        x2[:, kt, :], x2[:, kt, :], xo[:, kt, :], ALU.add)
                rsb = ln_stats(x2, DK, sq_eng=nc.gpsimd)
                x2s = mp.tile([P, DK, 512], BF16, tag="x2s", bufs=1)
                x2s_t[i] = x2s
                for kt in range(DK):
                    eng = nc.vector if kt % 2 == 0 else nc.gpsimd
                    eng.tensor_mul(x2s[:, kt, :], x2[:, kt, :], rsb)

            def fc1_piece(i, fs, fe, mdst, raw=False):
                """fc1 for global ffts [fs, fe) of chunk i into mdst (slot =
                fft % FH).  raw=True copies the pre-activation out on DVE
                instead of applying gelu, so the gelus can run as one batch
                later (ACT table-switch thrash vs the attention exp stream)."""
                x2s = x2s_t[i]
                for fft in range(fs, fe):
                    w1t = mp.tile([P, DK, P], BF16, tag="w1t", bufs=3)
                    nc.sync.dma_start(w1t, w1[fft])
                    pp = psum.tile([P, 512], FP32, tag="acc", bufs=2)
                    for kt in range(DK):
                        nc.tensor.matmul(pp, w1t[:, kt, :], x2s[:, kt, :],
                                         start=(kt == 0), stop=(kt == DK - 1))
                    if raw:
                        nc.vector.tensor_copy(mdst[:, fft % FH, :], pp)
                    else:
                        nc.scalar.activation(mdst[:, fft % FH, :], pp,
                                             AF.Gelu_apprx_tanh,
                                             bias=c1_sb[:, fft:fft + 1])

            def gelu_batch(mdst, fs, fe):
                """in-place gelu over raw fc1 slots for global ffts
                [fs, fe)."""
                for fft in range(fs, fe):
                    nc.scalar.activation(mdst[:, fft % FH, :],
                                         mdst[:, fft % FH, :],
                                         AF.Gelu_apprx_tanh,
                                         bias=c1_sb[:, fft:fft + 1])

            def fc2_half(i, half, msrc, acc2):
                """fc2 partial over ffts [half*FH, (half+1)*FH)."""
                x2 = x2_t[i]
                for ot in range(DK):
                    w2t = mp.tile([P, FH, P], BF16, tag="w2t", bufs=2)
                    nc.sync.dma_start(w2t, w2[ot, :, half * FH:(half + 1) * FH])
                    pp = psum.tile([P, 512], FP32, tag="acc", bufs=2)
                    for ff in range(FH):
                        nc.tensor.matmul(pp, w2t[:, ff, :],
                                         msrc[:, ff, :],
                                         start=(ff == 0), stop=(ff == FH - 1))
                    if half == 0:
                        # stash (fc2A + b2 + x2) in bf16
                        nc.vector.scalar_tensor_tensor(
                            acc2[:, ot, :], pp, b2_sb[:, ot:ot + 1],
                            x2[:, ot, :], ALU.add, ALU.add)
                    else:
                        ot_t = mp.tile([P, 512], FP32, tag="outt", bufs=1)
                        nc.vector.tensor_tensor(ot_t, pp, acc2[:, ot, :],
                                                ALU.add)
                        nc.sync.dma_start(
                            out[:, ot, i * 512:(i + 1) * 512], ot_t)

            def attn(qt, mlp_cb=None, tail_cb=None):
                """attention + Wo partial for q-tile qt.  mlp_cb(pt) lets the
                caller interleave MLP pieces between pt bands.  Softmax
                normalization is deferred: y and the denominator row are
                copied out of PSUM per head (bf16), then one batched
                reciprocal [8,512] per q-tile normalizes everything."""
                nkt = 4 * qt + 4
                qv = qcur_t[qt]
                ysb = mp.tile([P, NOT, 512], BF16, tag="ysb", bufs=2)
                ycop = mp.tile([HD + 1, 2 * NOT, 512], BF16, tag="ycop",
                               bufs=1)
                dent = mp.tile([2 * NOT, 512], BF16, tag="dent", bufs=1)
                for pt in range(NOT):
                    y_ps = [psum.tile([HD + 1, 512], FP32, name=f"yps{hb}",
                                      tag=f"y{hb}", bufs=1) for hb in range(2)]
                    for kt in range(nkt):
                        ksl = slice(kt * P, (kt + 1) * P)
                        sg = psum.tile([P, 1024], FP32, tag="sg", bufs=2)
                        nc.tensor.matmul(sg[:, 0:512],
                                         kT_sb[0:HD, pt, ksl],
                                         qv[0:HD, pt, :],
                                         start=True, stop=True)
                        nc.tensor.matmul(sg[:, 512:1024],
                                         kT_sb[HD:P, pt, ksl],
                                         qv[HD:P, pt, :],
                                         start=True, stop=True)
                        pe = mp.tile([P, 1024], BF16, tag="pexp", bufs=3)
                        nc.scalar.activation(pe, sg, AF.Exp, scale=SCALE)
                        j = kt - 4 * qt
                        if j >= 0:
                            moff = 384 - P * j
                            nc.vector.tensor_mul(
                                pe[:, 0:512], pe[:, 0:512],
                                masks_sb[:, moff:moff + 512])
                            nc.vector.tensor_mul(
                                pe[:, 512:1024], pe[:, 512:1024],
                                masks_sb[:, moff:moff + 512])
                        nc.tensor.matmul(y_ps[0], v_sb[:, 2 * pt, kt, :],
                                         pe[:, 0:512],
                                         start=(kt == 0), stop=(kt == nkt - 1))
                        nc.tensor.matmul(y_ps[1], v_sb[:, 2 * pt + 1, kt, :],
                                         pe[:, 512:1024],
                                         start=(kt == 0), stop=(kt == nkt - 1))
                    for hb in range(2):
                        j = 2 * pt + hb
                        nc.vector.tensor_copy(ycop[:, j, :], y_ps[hb])
                        nc.sync.dma_start(dent[j:j + 1, :],
                                          ycop[HD:HD + 1, j, :])
                    if mlp_cb is not None:
                        mlp_cb(pt)
                # batched softmax denominators for all 8 heads of this q-tile
                rcp = mp.tile([2 * NOT, 512], FP32, tag="rcp", bufs=1)
                nc.vector.reciprocal(rcp, dent)
                rcpb = mp.tile([2 * NOT, 512], BF16, tag="rcpb", bufs=1)
                nc.vector.tensor_copy(rcpb, rcp)
                for pt in range(NOT):
                    for hb in range(2):
                        j = 2 * pt + hb
                        rec = mp.tile([1, 512], BF16, tag="rec", bufs=1)
                        nc.sync.dma_start(rec, rcpb[j:j + 1, :])
                        rb = mp.tile([HD, 512], BF16, tag="rb", bufs=2)
                        nc.gpsimd.partition_broadcast(rb, rec)
                        if hb == 0:
                            nc.vector.tensor_mul(ysb[0:HD, pt, :],
                                                 ycop[0:HD, j, :], rb)
                        else:
                            yst = mp.tile([HD, 512], BF16, tag="yst", bufs=1)
                            nc.vector.tensor_mul(yst, ycop[0:HD, j, :], rb)
                            nc.sync.dma_start(ysb[HD:P, pt, :], yst)
                # Wo partials
                ast = mp.tile([P, DK, 512], BF16, tag="ast", bufs=1)
                for ot in range(DK):
                    pp = psum.tile([P, 512], FP32, tag="acc", bufs=2)
                    for pt in range(NOT):
                        nc.tensor.matmul(pp, wo_sb[:, pt, ot * P:(ot + 1) * P],
                                         ysb[:, pt, :],
                                         start=(pt == 0), stop=(pt == NOT - 1))
                    if ot % 2 == 0:
                        nc.vector.tensor_copy(ast[:, ot, :], pp)
                    else:
                        nc.scalar.copy(ast[:, ot, :], pp)
                nc.sync.dma_start(rs_in[qt // 2][qt % 2], ast)
                if qt % 2 == 1:
                    i = qt // 2
                    nc.gpsimd.collective_compute(
                        "ReduceScatter", ALU.add,
                        replica_groups=[[0, 1], [2, 3], [4, 5], [6, 7]],
                        ins=[rs_in[i].opt()], outs=[rs_out[i].opt()])
                if tail_cb is not None:
                    tail_cb()

            # ==================== program ====================
            # RS pairing: rs_in[0] = {qt0 (slot0), qt1 (slot1)} fires inside
            # attn(1); rs_in[1] = {qt2, qt3} fires inside attn(3).  Core h of
            # each pair owns tokens of q-tiles {h, h+2}, so chunk-0 MLP can
            # overlap attn(2)/attn(3) instead of running after them.
            cur = ln_part(0)
            for tt in range(NT):
                nxt = ln_part(tt + 1) if tt < NT - 1 else None
                phase_ab(tt, *cur)
                cur = nxt
                if 1 <= tt <= 2:
                    attn(tt - 1)     # attn(1) fires ReduceScatter #0
            m0a = mp.tile([P, FH, 512], BF16, tag="bigbuf", bufs=2,
                          name="m0a")
            m0b = mp.tile([P, FH, 512], BF16, tag="bigbuf", bufs=2,
                          name="m0b")
            acc2_0 = mp.tile([P, DK, 512], BF16, tag="acc2", bufs=1,
                             name="acc2_0")

            def mlp_a_cb(pt):
                if pt == 3:
                    # floor the schedule time past ReduceScatter #0's real
                    # completion so the scheduler doesn't hoist these into
                    # the collective's latency window (it underestimates it)
                    with tc.tile_wait_until(0.30):
                        phase_e(0)

            def mlp_b_cb(pt):
                if pt == 0:
                    fc1_piece(0, 0, 8, m0a, raw=True)
                elif pt == 1:
                    fc1_piece(0, 8, FH, m0a, raw=True)
                    gelu_batch(m0a, 0, FH)
                elif pt == 2:
                    fc2_half(0, 0, m0a, acc2_0)

            def mlp_b_tail():
                # runs right after ReduceScatter #1 fires: keeps the PE fed
                # while the collective is in flight.  m0b is a separate
                # buffer, so none of this waits on fc2_half(0,0)'s reads.
                fc1_piece(0, FH, 24, m0b, raw=True)
                gelu_batch(m0b, FH, 24)
                fc1_piece(0, 24, FFT, m0b, raw=True)
                gelu_batch(m0b, 24, FFT)
                fc2_half(0, 1, m0b, acc2_0)

            attn(2, mlp_cb=mlp_a_cb)
            attn(3, mlp_cb=mlp_b_cb, tail_cb=mlp_b_tail)  # fires RS #1
            # same scheduling floor as phase_e(0): keep the RS#1-dependent
            # LN2 ops from being hoisted ahead of the chunk-0 fc2 tail
            with tc.tile_wait_until(0.58):
                phase_e(1)
            m1a = mp.tile([P, FH, 512], BF16, tag="bigbuf", bufs=2,
                          name="m1a")
            m1b = mp.tile([P, FH, 512], BF16, tag="bigbuf", bufs=2,
                          name="m1b")
            acc2_1 = mp.tile([P, DK, 512], BF16, tag="acc2", bufs=1,
                             name="acc2_1")
            fc1_piece(1, 0, FH, m1a)
            fc1_piece(1, FH, FFT, m1b)
            fc2_half(1, 0, m1a, acc2_1)
            fc2_half(1, 1, m1b, acc2_1)

    nc.compile()
    return nc


_NC_CACHE = None


def _get_nc():
    global _NC_CACHE
    if _NC_CACHE is None:
        _NC_CACHE = build_program()
    return _NC_CACHE


def prep_in_maps(x, ln1_g, ln1_b, ln2_g, ln2_b, Wq, bq, Wk, bk, Wv, bv,
                 Wo, bo, W1, b1, W2, b2):
    import ml_dtypes
    f32 = np.float32
    bf16 = ml_dtypes.bfloat16
    x = np.asarray(x, f32)
    ln1_g, ln1_b = np.asarray(ln1_g, f32), np.asarray(ln1_b, f32)
    ln2_g, ln2_b = np.asarray(ln2_g, f32), np.asarray(ln2_b, f32)
    Wq, Wk, Wv, Wo = (np.asarray(a, f32) for a in (Wq, Wk, Wv, Wo))
    W1, W2 = np.asarray(W1, f32), np.asarray(W2, f32)
    bq, bk, bv, bo_, b1, b2_ = (np.asarray(a, f32)
                                for a in (bq, bk, bv, bo, b1, b2))

    # fold LN gain AND the mean subtraction (a rank-1 correction) into W:
    # (x - mu) * g @ W  =  x @ (g*W - colsum(g*W)/D)
    Wqg = ln1_g[:, None] * Wq
    Wkg = ln1_g[:, None] * Wk
    Wvg = ln1_g[:, None] * Wv
    Wqg = Wqg - Wqg.sum(0, keepdims=True) / D
    Wkg = Wkg - Wkg.sum(0, keepdims=True) / D
    Wvg = Wvg - Wvg.sum(0, keepdims=True) / D
    cq_full = ln1_b @ Wq + bq
    ck_full = ln1_b @ Wk + bk
    cv_full = ln1_b @ Wv + bv
    W1g = ln2_g[:, None] * W1
    W1g = W1g - W1g.sum(0, keepdims=True) / D
    c1_full = ln2_b @ W1 + b1

    w1_t = np.ascontiguousarray(
        W1g.reshape(DK, P, FFT, P).transpose(2, 1, 0, 3)).astype(bf16)
    w2_t = np.ascontiguousarray(
        W2.reshape(FFT, P, DK, P).transpose(2, 1, 0, 3)).astype(bf16)
    c1_t = np.ascontiguousarray(c1_full.reshape(FFT, P).T)      # [P,FFT]
    b2_t = np.ascontiguousarray(b2_.reshape(DK, P).T)           # [P,DK]

    kk = np.arange(P)[:, None]
    cc = np.arange(896)[None, :]
    mk = (kk + 384 <= cc).astype(bf16)                          # [P,896]

    in_maps = []
    for c in range(8):
        b_idx, hh = c // 2, c % 2
        sl = slice(DQ * hh, DQ * hh + DQ)
        xT_c = np.ascontiguousarray(
            x[b_idx].T.reshape(DK, P, T).transpose(1, 0, 2)).astype(bf16)
        wq_c, wk_c = Wqg[:, sl], Wkg[:, sl]
        wqk_t = np.ascontiguousarray(
            np.stack([wq_c, wk_c]).reshape(2, DK, P, NOT, P)
            .transpose(2, 0, 3, 1, 4)).astype(bf16)             # [P,2,NOT,DK,P]
        wv_t = np.ascontiguousarray(
            Wvg[:, sl].reshape(DK, P, DQ).transpose(1, 0, 2)).astype(bf16)
        wo_t = np.ascontiguousarray(
            Wo[sl, :].reshape(NOT, P, D).transpose(1, 0, 2)).astype(bf16)
        cq_t = cq_full[sl].reshape(NOT, P).T                    # [P,NOT]
        ck_t = ck_full[sl].reshape(NOT, P).T
        xobo = np.ascontiguousarray(
            (x[b_idx] + bo_).T.reshape(DK, P, T).transpose(1, 0, 2)
        ).astype(bf16)
        own = np.concatenate(
            [xobo[:, :, hh * 512:(hh + 1) * 512],
             xobo[:, :, (hh + 2) * 512:(hh + 3) * 512]], axis=2)
        in_maps.append({
            "xT": xT_c,
            "xTo": np.ascontiguousarray(own),
            "wqk": wqk_t,
            "wv": wv_t,
            "wo": wo_t,
            "w1": w1_t,
            "w2": w2_t,
            "cqk": np.ascontiguousarray(np.concatenate([cq_t, ck_t], axis=1)),
            "cvb": np.broadcast_to(
                cv_full[sl][None, :], (P, DQ)).astype(bf16),
            "c1": c1_t,
            "b2": b2_t,
            "masks": mk,
        })
    return in_maps


def assemble_output(results):
    out = np.empty((B, T, D), np.float32)
    for c in range(8):
        b_idx, hh = c // 2, c % 2
        o = results[c]["out"]                                   # [P,DK,TOWN]
        full = o.transpose(2, 1, 0).reshape(TOWN, D)
        out[b_idx, hh * 512:(hh + 1) * 512, :] = full[0:512]
        out[b_idx, (hh + 2) * 512:(hh + 3) * 512, :] = full[512:1024]
    return out


def kernel(**inputs):
    nc = _get_nc()
    in_maps = prep_in_maps(**inputs)
    res = run_bass_kernel_spmd(nc, in_maps, list(range(8)))
    return assemble_output(res.results)

